# revision 37
# baseline (speedup 1.0000x reference)
"""BitConv2d (ternary-weight 3x3 conv, power-of-two rescale) on 8 TRN2 NeuronCores.

Strategy:
  - Data-parallel over batch: 32 images -> 4 per core (2 image pairs).
  - Activation quantization x_int = clip(round(clip(x,-1,1)/2^-6), -127, 127)
    computed exactly: i16 = RNE(64*x + 128) on GpSimd (hw f32->i16 cast),
    bf16 = clip(i16, 64, 192) on DVE -> v = x_int + 128 (exact ints in bf16).
    Padded border cells are memset to 128 so the offset contributes exactly
    128*sum(w) per output channel, folded into the bias on the host.
  - Conv as accumulating 64x64-quadrant matmuls (K=Cin=64, M=Cout=64),
    4-per-array via tile_position quadrants:
      rows 0-63 = image (2k) channels, rows 64-127 = image (2k+1),
      cols 0-63 = psum parts 0-63  (output row-block r),
      cols 64-127 = psum parts 64-127 (row-block r+1).
    Each psum bank [128, 448] holds ONE image's two row-blocks; weight loads
    are amortized over SWEEP=2 iterations (8 matmuls per 4 quadrant loads).
  - Epilogue y = psum * 2^(act_exp+s_exp[c]) + bias'[c] in one full-width
    [128,448] ACT op per image per iter, written as bf16 into a small
    per-(sweep, image) buffer laid out so each partition's data is one
    contiguous DRAM segment (1 descriptor/partition), stored immediately.
  - The activation image is quantized into 7 slice tiles per pair (16 output
    rows + 2-row halo each); sweep s touches only slice s, so conv starts as
    soon as the first ~17 input rows are quantized.
  - I/O: 16 input DMAs (16-row chunks, [128p x 7KB]) and 28 output DMAs
    ([128p x 1.8KB] bf16); the HWDGE queue cost is mostly per-DMA, so few
    big-descriptor DMAs.  Output layout [img][p=(blk,ch)][it*448 + r*112 + w]
    with output row = 8*it + 4*blk + r, decoded on the host.
"""

import numpy as np
import ml_dtypes
from contextlib import ExitStack

_NC_CACHE = {}

N_CORES = 8
H = W = 112
HP = H + 2  # padded
CIN = COUT = 64
P = 128
IMGS_PER_CORE = 4
ROWS_PER_TILE = 4             # output rows per matmul tile (N = 4*112 = 448)
NFREE = ROWS_PER_TILE * W     # 448
N_ITERS = 14                  # 8-row output iterations per image pair
N_SLICES = 7                  # xq slices per pair; slice s <-> sweep s
SLICE_SLOTS = 18              # padded rows per slice (16 + 2-row halo)
OBUF_N = N_ITERS * NFREE      # 6272 bf16 per partition per image


def _patch_tile_drain(tile_mod):
    """This walrus build rejects a Drain carrying many sync waits; split the
    final Tile drain into single-wait sync nops."""
    from concourse.vector_clock import ScopedClock, VectorClock

    if getattr(tile_mod.TileContext, "_drain_patched", False):
        return

    def _drain_and_barrier_split(self, tick_clock, wait_clock):
        vclock = tick_clock.global_clock
        n = len(vclock)
        for proc in range(n):
            t = vclock[proc]
            if t <= 0:
                continue
            vec = [0] * n
            vec[proc] = t
            nop = self.nc.sync.nop()
            wait_clock.add_sem_waits(nop.ins, ScopedClock({None: VectorClock(vec)}))
        self.nc.sync.drain()
        assert self.sems is not None
        popped = self.nc._tile_sem_poison_stack.pop()
        assert popped is self._sem_poison
        self.nc.all_engine_barrier()
        self.nc.clear_and_free_semaphores(list(self.sems.allocated().values()))
        self.nc.all_engine_barrier()

    tile_mod.TileContext._drain_and_barrier = _drain_and_barrier_split
    tile_mod.TileContext._drain_patched = True


def _split_multi_syncs(nc):
    """This walrus build accepts at most ONE sync wait (and one update) per
    instruction.  Hoist extra waits onto preceding nops and extra updates onto
    following nops (same engine, so ordering semantics are preserved)."""
    import concourse.mybir as mybir

    fn = nc.m.functions[0]
    ctr = 0
    for bb in fn.blocks:
        new_insts = []
        for inst in bb.instructions:
            si = inst.sync_info
            pre, post = [], []
            if si is not None and si.on_wait and len(si.on_wait) > 1:
                for w in list(si.on_wait[:-1]):
                    ctr += 1
                    pre.append(
                        mybir.InstNoOp(
                            name=f"wsplit_nop_{ctr}",
                            engine=inst.engine,
                            sync_info=mybir.SyncInfo(on_wait=[w], on_update=[]),
                        )
                    )
                si.on_wait = [si.on_wait[-1]]
            if (
                si is not None
                and si.on_update
                and len(si.on_update) > 1
                and not isinstance(inst, (mybir.InstDMACopy, mybir.InstDMA))
            ):
                for u in list(si.on_update[1:]):
                    ctr += 1
                    post.append(
                        mybir.InstNoOp(
                            name=f"usplit_nop_{ctr}",
                            engine=inst.engine,
                            sync_info=mybir.SyncInfo(on_wait=[], on_update=[u]),
                        )
                    )
                si.on_update = [si.on_update[0]]
            new_insts.extend(pre)
            new_insts.append(inst)
            new_insts.extend(post)
        if len(new_insts) != len(bb.instructions):
            bb.instructions[:] = new_insts
    for bb in fn.blocks:
        for inst in bb.instructions:
            if inst.name.startswith(("wsplit_nop_", "usplit_nop_")):
                if inst.name not in nc.inst_map:
                    nc.register_instruction(inst)
    return ctr


def _fuse_tap_ldweights(nc):
    """Tile emits one InstLdweights per matmul.  Within a tap-sweep the same
    four quadrant loads repeat for every iteration; delete an InstLdweights
    when the identical load (same tile_position, tensor, offset) is already
    resident in that quadrant, and mark every matmul ldweights=False so
    walrus doesn't re-synthesize loads for the now-bare matmuls.  Sync
    waits/updates of deleted loads are preserved on PE nops."""
    import concourse.mybir as mybir

    fn = nc.m.functions[0]
    ctr = 0
    n_deleted = 0
    for bb in fn.blocks:
        new = []
        resident = {}        # tile_position -> (memref, offset)
        changed = False
        for inst in bb.instructions:
            if isinstance(inst, mybir.InstLdweights):
                pos = tuple(inst.tile_position or (0, 0))
                ap = inst.ins[0]
                key = (ap.memref, ap.offset, tuple(tuple(d) for d in ap.ap))
                if resident.get(pos) == key:
                    si = inst.sync_info
                    if si is not None:
                        for w_ in list(si.on_wait or []):
                            ctr += 1
                            new.append(mybir.InstNoOp(
                                name=f"ldwfuse_w_{ctr}", engine=inst.engine,
                                sync_info=mybir.SyncInfo(
                                    on_wait=[w_], on_update=[]),
                            ))
                        for u in list(si.on_update or []):
                            ctr += 1
                            new.append(mybir.InstNoOp(
                                name=f"ldwfuse_u_{ctr}", engine=inst.engine,
                                sync_info=mybir.SyncInfo(
                                    on_wait=[], on_update=[u]),
                            ))
                    changed = True
                    n_deleted += 1
                    continue
                resident[pos] = key
            elif isinstance(inst, mybir.InstMatmult):
                inst.ldweights = False
            elif isinstance(inst, mybir.InstDrain):
                resident.clear()
            new.append(inst)
        if changed:
            bb.instructions[:] = new
    for bb in fn.blocks:
        for inst in bb.instructions:
            if inst.name.startswith(("ldwfuse_w_", "ldwfuse_u_")):
                if inst.name not in nc.inst_map:
                    nc.register_instruction(inst)
    return n_deleted


def build_nc():
    import concourse.bass as bass
    import concourse.mybir as mybir
    import concourse.tile as tile

    _patch_tile_drain(tile)

    f32 = mybir.dt.float32
    bf16 = mybir.dt.bfloat16
    i16 = mybir.dt.int16
    Alu = mybir.AluOpType
    Act = mybir.ActivationFunctionType

    nc = bass.Bass(trn_type="TRN2")
    xin = nc.dram_tensor("xin", (2 * P, H, W), f32, kind="ExternalInput")
    wsb = nc.dram_tensor("wsb", (P, 9 * P), bf16, kind="ExternalInput")
    sb = nc.dram_tensor("sb", (P, 2), f32, kind="ExternalInput")
    yout = nc.dram_tensor(
        "yout", (IMGS_PER_CORE, P, OBUF_N), bf16, kind="ExternalOutput"
    )

    n_pairs = IMGS_PER_CORE // 2

    # quant chunk c supplies everything slice c still needs (chunk c done =>
    # slice c complete); chunk 0 is split for a faster first slice.
    CHUNKS = [(0, 7), (8, 16)] + [
        (16 * c + 1, min(16 * c + 16, H - 1)) for c in range(1, N_SLICES)
    ]
    CHUNKS_FOR_SWEEP = [[0, 1], [2], [3], [4], [5], [6], [7]]

    with tile.TileContext(nc) as tc, ExitStack() as ctx:
        const_pool = ctx.enter_context(tc.tile_pool(name="const", bufs=1))
        xq_pool = ctx.enter_context(tc.tile_pool(name="xq", bufs=2 * N_SLICES))
        stg_pool = ctx.enter_context(tc.tile_pool(name="stg", bufs=5))
        rnd_pool = ctx.enter_context(tc.tile_pool(name="rnd", bufs=5))
        out_pool = ctx.enter_context(tc.tile_pool(name="out", bufs=6))
        psum_pool = ctx.enter_context(
            tc.tile_pool(name="psum", bufs=8, space=bass.MemorySpace.PSUM)
        )

        w_t = const_pool.tile([P, 9 * P], bf16)
        sb_t = const_pool.tile([P, 2], f32)

        def alloc_slices():
            # slice s holds padded rows 16s .. 16s+17 (2-row halo between
            # consecutive slices); sweep s reads only slice s.
            sl = [xq_pool.tile([P, SLICE_SLOTS, HP], bf16, name="xq")
                  for _ in range(N_SLICES)]
            for s, t in enumerate(sl):
                nc.vector.memset(t[:, :, 0], 128.0)
                nc.vector.memset(t[:, :, HP - 1], 128.0)
            nc.vector.memset(sl[0][:, 0, :], 128.0)
            nc.vector.memset(sl[-1][:, SLICE_SLOTS - 1, :], 128.0)
            return sl

        def emit_quant(pr, slices, ch):
            r0, r1 = CHUNKS[ch]
            nrows = r1 - r0 + 1
            stg = stg_pool.tile([P, nrows, W], f32, name="stg")
            nc.sync.dma_start(stg[:], xin[pr * P:(pr + 1) * P, r0:r1 + 1, :])
            rnd = rnd_pool.tile([P, nrows, W], i16, name="rnd")
            # i16 = RNE(64*x + 128): hw f32->i16 cast rounds to nearest even
            nc.gpsimd.tensor_scalar(out=rnd[:], in0=stg[:], scalar1=64.0,
                                    scalar2=128.0, op0=Alu.mult, op1=Alu.add)
            # bf16 = clip(i16, 64, 192) == x_int + 128 into the slice tiles
            # (padded rows r0+1 .. r1+1; slice s spans 16s .. 16s+17)
            p0, p1 = r0 + 1, r1 + 1
            for s in range(N_SLICES):
                lo, hi = max(p0, 16 * s), min(p1, 16 * s + SLICE_SLOTS - 1)
                if lo > hi:
                    continue
                nc.vector.tensor_scalar(
                    out=slices[s][:, lo - 16 * s:hi - 16 * s + 1, 1:1 + W],
                    in0=rnd[:, lo - p0:hi - p0 + 1, :],
                    scalar1=64, scalar2=192, op0=Alu.max, op1=Alu.min,
                )

        def emit_sweep(pr, slices, s):
            """Conv iters 2s, 2s+1 from slice s.  Per (iter, image) one psum
            bank holds both row-blocks; redundant quadrant weight reloads are
            stripped afterwards by _fuse_tap_ldweights."""
            its = (2 * s, 2 * s + 1)
            xq = slices[s]
            ps = {}
            for it in its:
                for im in range(2):
                    ps[(it, im)] = psum_pool.tile([P, NFREE], f32, name="ps")
            for tap in range(9):
                dh, dw = divmod(tap, 3)
                first, last = tap == 0, tap == 8
                for it in its:
                    base = it * 8 - 16 * s
                    for im, r in ((0, 0), (1, 64)):
                        for blk in range(2):
                            hs = base + ROWS_PER_TILE * blk + dh
                            c = 64 * blk
                            nc.tensor.matmul(
                                ps[(it, im)][c:c + 64, :],
                                w_t[r:r + 64, tap * P + c:tap * P + c + 64],
                                xq[r:r + 64, hs:hs + ROWS_PER_TILE, dw:dw + W],
                                start=first, stop=last,
                            )
            # epilogue: one full-width op per (iter, image) -- ACT for image
            # a, DVE for image b (parallel drains) -- bf16 out into a small
            # per-(sweep, image) buffer, stored immediately.
            for im in range(2):
                ob = out_pool.tile([P, 2 * NFREE], bf16, name="ob")
                for j, it in enumerate(its):
                    dst = ob[:, j * NFREE:(j + 1) * NFREE]
                    if im == 0:
                        nc.scalar.activation(
                            dst, ps[(it, im)][:], Act.Identity,
                            scale=sb_t[:, 0:1], bias=sb_t[:, 1:2],
                        )
                    else:
                        nc.vector.tensor_scalar(
                            out=dst, in0=ps[(it, im)][:],
                            scalar1=sb_t[:, 0:1], scalar2=sb_t[:, 1:2],
                            op0=Alu.mult, op1=Alu.add,
                        )
                img = 2 * pr + im
                nc.sync.dma_start(
                    yout[img, :, its[0] * NFREE:(its[-1] + 1) * NFREE], ob[:]
                )

        # software pipeline: conv(pair k) interleaves with quant(pair k+1).
        # First input chunk's DMA goes ahead of the weight DMAs on the queue.
        slices_k = alloc_slices()
        emit_quant(0, slices_k, 0)
        nc.sync.dma_start(w_t[:], wsb[:])
        nc.sync.dma_start(sb_t[:], sb[:])
        for ch in range(1, len(CHUNKS)):
            emit_quant(0, slices_k, ch)
        for pr in range(n_pairs):
            slices_next = alloc_slices() if pr + 1 < n_pairs else None
            for s in range(N_SLICES):
                if slices_next is not None:
                    for ch in CHUNKS_FOR_SWEEP[s]:
                        emit_quant(pr + 1, slices_next, ch)
                emit_sweep(pr, slices_k, s)
            slices_k = slices_next

    _fuse_tap_ldweights(nc)
    _split_multi_syncs(nc)
    nc.finalize()
    return nc


def _host_prep(w_q, s_exp, bias, act_exp):
    """Weights: per tap a [128,128] block = 64x64 [cin,cout] duplicated 2x2
    (rows: image halves, cols: row-block halves).  Scale/bias fold."""
    w_q = np.asarray(w_q)
    w1 = np.transpose(w_q.reshape(COUT, CIN, 9), (1, 2, 0))       # [ci, t, co]
    w2 = np.concatenate([w1, w1], axis=2)                         # [ci, t, 128]
    w2 = w2.reshape(CIN, 9 * P)
    wsb = np.concatenate([w2, w2], axis=0).astype(ml_dtypes.bfloat16)

    s_exp = np.asarray(s_exp).reshape(-1).astype(np.float64)
    scale = np.exp2(float(act_exp) + s_exp)                       # [64]
    wsum = w_q.astype(np.float64).sum(axis=(1, 2, 3))             # [64]
    bias_c = np.asarray(bias).astype(np.float64) - 128.0 * wsum * scale
    col_scale = np.tile(scale, 2).astype(np.float32)
    col_bias = np.tile(bias_c, 2).astype(np.float32)
    sb = np.stack([col_scale, col_bias], axis=1)                  # [128, 2] f32
    return wsb, sb


def _decode_out(y):
    """[4, 128, 6272] bf16 -> [4, 64, 112, 112] f32.
    p = blk*64 + ch; free = it*448 + r*112 + w;
    output row = 8*it + 4*blk + r."""
    y = np.asarray(y).astype(np.float32)
    y = y.reshape(IMGS_PER_CORE, 2, CIN, N_ITERS, ROWS_PER_TILE, W)
    #            img            blk  ch   it       r             w
    y = np.transpose(y, (0, 2, 3, 1, 4, 5))      # img ch it blk r w
    return y.reshape(IMGS_PER_CORE, COUT, H, W)


def kernel(x, w_q, s_exp, bias, act_exp):
    from concourse.bass_utils import run_bass_kernel_spmd

    x = np.ascontiguousarray(np.asarray(x, dtype=np.float32))
    wsb, sb = _host_prep(np.asarray(w_q), s_exp, bias, int(act_exp))

    if "nc" not in _NC_CACHE:
        _NC_CACHE["nc"] = build_nc()
    nc = _NC_CACHE["nc"]

    in_maps = [
        {"xin": x[4 * c:4 * c + 4].reshape(2 * P, H, W), "wsb": wsb, "sb": sb}
        for c in range(N_CORES)
    ]
    _NC_CACHE["in_maps"] = in_maps
    res = run_bass_kernel_spmd(nc, in_maps, core_ids=list(range(N_CORES)))
    out = np.concatenate(
        [_decode_out(res.results[c]["yout"]) for c in range(N_CORES)], axis=0
    )
    return np.ascontiguousarray(out, dtype=np.float32)


# revision 39
# speedup vs baseline: 1.1806x; 1.1806x over previous
"""BitConv2d (ternary-weight 3x3 conv, power-of-two rescale) on 8 TRN2 NeuronCores.

Strategy:
  - Data-parallel over batch: 32 images -> 4 per core (2 image pairs).
  - Activation quantization x_int = clip(round(clip(x,-1,1)/2^-6), -127, 127)
    computed exactly: i16 = RNE(64*x + 128) on GpSimd (hw f32->i16 cast),
    bf16 = clip(i16, 64, 192) on DVE -> v = x_int + 128 (exact ints in bf16).
    Padded border cells are memset to 128 so the offset contributes exactly
    128*sum(w) per output channel, folded into the bias on the host.
  - Conv as accumulating 64x64-quadrant matmuls (K=Cin=64, M=Cout=64),
    4-per-array via tile_position quadrants:
      rows 0-63 = image (2k) channels, rows 64-127 = image (2k+1),
      cols 0-63 = psum parts 0-63  (output row-block r),
      cols 64-127 = psum parts 64-127 (row-block r+1).
    Each psum bank [128, 448] holds ONE image's two row-blocks; weight loads
    are amortized over SWEEP=2 iterations (8 matmuls per 4 quadrant loads).
  - Epilogue y = psum * 2^(act_exp+s_exp[c]) + bias'[c] in one full-width
    [128,448] ACT op per image per iter, written as bf16 into a small
    per-(sweep, image) buffer laid out so each partition's data is one
    contiguous DRAM segment (1 descriptor/partition), stored immediately.
  - The activation image is quantized into 7 slice tiles per pair (16 output
    rows + 2-row halo each); sweep s touches only slice s, so conv starts as
    soon as the first ~17 input rows are quantized.
  - I/O: 16 input DMAs (16-row chunks, [128p x 7KB]) and 28 output DMAs
    ([128p x 1.8KB] bf16); the HWDGE queue cost is mostly per-DMA, so few
    big-descriptor DMAs.  Output layout [img][p=(blk,ch)][it*448 + r*112 + w]
    with output row = 8*it + 4*blk + r, decoded on the host.
"""

import numpy as np
import ml_dtypes
from contextlib import ExitStack

_NC_CACHE = {}

N_CORES = 8
H = W = 112
HP = H + 2  # padded
CIN = COUT = 64
P = 128
IMGS_PER_CORE = 4
ROWS_PER_TILE = 4             # output rows per matmul tile (N = 4*112 = 448)
NFREE = ROWS_PER_TILE * W     # 448
N_ITERS = 14                  # 8-row output iterations per image pair
N_SLICES = 7                  # xq slices per pair; slice s <-> sweep s
SLICE_SLOTS = 18              # padded rows per slice (16 + 2-row halo)
OBUF_N = N_ITERS * NFREE      # 6272 bf16 per partition per image


def _patch_tile_drain(tile_mod):
    """This walrus build rejects a Drain carrying many sync waits; split the
    final Tile drain into single-wait sync nops."""
    from concourse.vector_clock import ScopedClock, VectorClock

    if getattr(tile_mod.TileContext, "_drain_patched", False):
        return

    def _drain_and_barrier_split(self, tick_clock, wait_clock):
        vclock = tick_clock.global_clock
        n = len(vclock)
        for proc in range(n):
            t = vclock[proc]
            if t <= 0:
                continue
            vec = [0] * n
            vec[proc] = t
            nop = self.nc.sync.nop()
            wait_clock.add_sem_waits(nop.ins, ScopedClock({None: VectorClock(vec)}))
        self.nc.sync.drain()
        assert self.sems is not None
        popped = self.nc._tile_sem_poison_stack.pop()
        assert popped is self._sem_poison
        self.nc.all_engine_barrier()
        self.nc.clear_and_free_semaphores(list(self.sems.allocated().values()))
        self.nc.all_engine_barrier()

    tile_mod.TileContext._drain_and_barrier = _drain_and_barrier_split
    tile_mod.TileContext._drain_patched = True


def _split_multi_syncs(nc):
    """This walrus build accepts at most ONE sync wait (and one update) per
    instruction.  Hoist extra waits onto preceding nops and extra updates onto
    following nops (same engine, so ordering semantics are preserved)."""
    import concourse.mybir as mybir

    fn = nc.m.functions[0]
    ctr = 0
    for bb in fn.blocks:
        new_insts = []
        for inst in bb.instructions:
            si = inst.sync_info
            pre, post = [], []
            if si is not None and si.on_wait and len(si.on_wait) > 1:
                for w in list(si.on_wait[:-1]):
                    ctr += 1
                    pre.append(
                        mybir.InstNoOp(
                            name=f"wsplit_nop_{ctr}",
                            engine=inst.engine,
                            sync_info=mybir.SyncInfo(on_wait=[w], on_update=[]),
                        )
                    )
                si.on_wait = [si.on_wait[-1]]
            if (
                si is not None
                and si.on_update
                and len(si.on_update) > 1
                and not isinstance(inst, (mybir.InstDMACopy, mybir.InstDMA))
            ):
                for u in list(si.on_update[1:]):
                    ctr += 1
                    post.append(
                        mybir.InstNoOp(
                            name=f"usplit_nop_{ctr}",
                            engine=inst.engine,
                            sync_info=mybir.SyncInfo(on_wait=[], on_update=[u]),
                        )
                    )
                si.on_update = [si.on_update[0]]
            new_insts.extend(pre)
            new_insts.append(inst)
            new_insts.extend(post)
        if len(new_insts) != len(bb.instructions):
            bb.instructions[:] = new_insts
    for bb in fn.blocks:
        for inst in bb.instructions:
            if inst.name.startswith(("wsplit_nop_", "usplit_nop_")):
                if inst.name not in nc.inst_map:
                    nc.register_instruction(inst)
    return ctr


def _fuse_tap_ldweights(nc):
    """Tile emits one InstLdweights per matmul.  Within a tap-sweep the same
    four quadrant loads repeat for every iteration; delete an InstLdweights
    when the identical load (same tile_position, tensor, offset) is already
    resident in that quadrant, and mark every matmul ldweights=False so
    walrus doesn't re-synthesize loads for the now-bare matmuls.  Sync
    waits/updates of deleted loads are preserved on PE nops."""
    import concourse.mybir as mybir

    fn = nc.m.functions[0]
    ctr = 0
    n_deleted = 0
    for bb in fn.blocks:
        new = []
        resident = {}        # tile_position -> (memref, offset)
        changed = False
        for inst in bb.instructions:
            if isinstance(inst, mybir.InstLdweights):
                pos = tuple(inst.tile_position or (0, 0))
                ap = inst.ins[0]
                key = (ap.memref, ap.offset, tuple(tuple(d) for d in ap.ap))
                if resident.get(pos) == key:
                    si = inst.sync_info
                    if si is not None:
                        for w_ in list(si.on_wait or []):
                            ctr += 1
                            new.append(mybir.InstNoOp(
                                name=f"ldwfuse_w_{ctr}", engine=inst.engine,
                                sync_info=mybir.SyncInfo(
                                    on_wait=[w_], on_update=[]),
                            ))
                        for u in list(si.on_update or []):
                            ctr += 1
                            new.append(mybir.InstNoOp(
                                name=f"ldwfuse_u_{ctr}", engine=inst.engine,
                                sync_info=mybir.SyncInfo(
                                    on_wait=[], on_update=[u]),
                            ))
                    changed = True
                    n_deleted += 1
                    continue
                resident[pos] = key
            elif isinstance(inst, mybir.InstMatmult):
                inst.ldweights = False
            elif isinstance(inst, mybir.InstDrain):
                resident.clear()
            new.append(inst)
        if changed:
            bb.instructions[:] = new
    for bb in fn.blocks:
        for inst in bb.instructions:
            if inst.name.startswith(("ldwfuse_w_", "ldwfuse_u_")):
                if inst.name not in nc.inst_map:
                    nc.register_instruction(inst)
    return n_deleted


def build_nc():
    import concourse.bass as bass
    import concourse.mybir as mybir
    import concourse.tile as tile

    _patch_tile_drain(tile)

    f32 = mybir.dt.float32
    bf16 = mybir.dt.bfloat16
    i16 = mybir.dt.int16
    Alu = mybir.AluOpType
    Act = mybir.ActivationFunctionType

    nc = bass.Bass(trn_type="TRN2")
    xin = nc.dram_tensor("xin", (2 * P, H, W), f32, kind="ExternalInput")
    wsb = nc.dram_tensor("wsb", (P, 9 * P), bf16, kind="ExternalInput")
    sb = nc.dram_tensor("sb", (P, 2), f32, kind="ExternalInput")
    yout = nc.dram_tensor(
        "yout", (IMGS_PER_CORE, P, OBUF_N), bf16, kind="ExternalOutput"
    )

    n_pairs = IMGS_PER_CORE // 2

    # quant chunk c supplies everything slice c still needs (chunk c done =>
    # slice c complete); chunk 0 is split for a faster first slice.
    CHUNKS = [(0, 7), (8, 16)] + [
        (16 * c + 1, min(16 * c + 16, H - 1)) for c in range(1, N_SLICES)
    ]
    CHUNKS_FOR_SWEEP = [[0, 1], [2], [3], [4], [5], [6], [7]]

    with tile.TileContext(nc) as tc, ExitStack() as ctx:
        const_pool = ctx.enter_context(tc.tile_pool(name="const", bufs=1))
        xq_pool = ctx.enter_context(tc.tile_pool(name="xq", bufs=2 * N_SLICES))
        stg_pool = ctx.enter_context(tc.tile_pool(name="stg", bufs=8))
        rnd_pool = ctx.enter_context(tc.tile_pool(name="rnd", bufs=8))
        out_pool = ctx.enter_context(tc.tile_pool(name="out", bufs=6))
        psum_pool = ctx.enter_context(
            tc.tile_pool(name="psum", bufs=8, space=bass.MemorySpace.PSUM)
        )

        w_t = const_pool.tile([P, 9 * P], bf16)
        sb_t = const_pool.tile([P, 2], f32)

        def alloc_slices():
            # slice s holds padded rows 16s .. 16s+17 (2-row halo between
            # consecutive slices); sweep s reads only slice s.
            sl = [xq_pool.tile([P, SLICE_SLOTS, HP], bf16, name="xq")
                  for _ in range(N_SLICES)]
            for s, t in enumerate(sl):
                nc.vector.memset(t[:, :, 0], 128.0)
                nc.vector.memset(t[:, :, HP - 1], 128.0)
            nc.vector.memset(sl[0][:, 0, :], 128.0)
            nc.vector.memset(sl[-1][:, SLICE_SLOTS - 1, :], 128.0)
            return sl

        def emit_quant(pr, slices, ch):
            r0, r1 = CHUNKS[ch]
            nrows = r1 - r0 + 1
            stg = stg_pool.tile([P, nrows, W], f32, name="stg")
            nc.sync.dma_start(stg[:], xin[pr * P:(pr + 1) * P, r0:r1 + 1, :])
            rnd = rnd_pool.tile([P, nrows, W], i16, name="rnd")
            # i16 = RNE(64*x + 128): hw f32->i16 cast rounds to nearest even
            nc.gpsimd.tensor_scalar(out=rnd[:], in0=stg[:], scalar1=64.0,
                                    scalar2=128.0, op0=Alu.mult, op1=Alu.add)
            # bf16 = clip(i16, 64, 192) == x_int + 128 into the slice tiles
            # (padded rows r0+1 .. r1+1; slice s spans 16s .. 16s+17)
            p0, p1 = r0 + 1, r1 + 1
            for s in range(N_SLICES):
                lo, hi = max(p0, 16 * s), min(p1, 16 * s + SLICE_SLOTS - 1)
                if lo > hi:
                    continue
                nc.vector.tensor_scalar(
                    out=slices[s][:, lo - 16 * s:hi - 16 * s + 1, 1:1 + W],
                    in0=rnd[:, lo - p0:hi - p0 + 1, :],
                    scalar1=64, scalar2=192, op0=Alu.max, op1=Alu.min,
                )

        def emit_sweep(pr, slices, s):
            """Conv iters 2s, 2s+1 from slice s.  Per (iter, image) one psum
            bank holds both row-blocks; redundant quadrant weight reloads are
            stripped afterwards by _fuse_tap_ldweights."""
            its = (2 * s, 2 * s + 1)
            xq = slices[s]
            ps = {}
            for it in its:
                for im in range(2):
                    ps[(it, im)] = psum_pool.tile([P, NFREE], f32, name="ps")
            for tap in range(9):
                dh, dw = divmod(tap, 3)
                first, last = tap == 0, tap == 8
                for it in its:
                    base = it * 8 - 16 * s
                    for im, r in ((0, 0), (1, 64)):
                        for blk in range(2):
                            hs = base + ROWS_PER_TILE * blk + dh
                            c = 64 * blk
                            nc.tensor.matmul(
                                ps[(it, im)][c:c + 64, :],
                                w_t[r:r + 64, tap * P + c:tap * P + c + 64],
                                xq[r:r + 64, hs:hs + ROWS_PER_TILE, dw:dw + W],
                                start=first, stop=last,
                            )
            # epilogue: one full-width ACT op per (iter, image), bf16 out
            # into a small per-(sweep, image) buffer, stored immediately.
            # (DVE is kept free for the quant clips -- putting epilogues
            # there stalls the slice supply chain.)
            for im in range(2):
                ob = out_pool.tile([P, 2 * NFREE], bf16, name="ob")
                for j, it in enumerate(its):
                    nc.scalar.activation(
                        ob[:, j * NFREE:(j + 1) * NFREE], ps[(it, im)][:],
                        Act.Identity, scale=sb_t[:, 0:1], bias=sb_t[:, 1:2],
                    )
                img = 2 * pr + im
                nc.sync.dma_start(
                    yout[img, :, its[0] * NFREE:(its[-1] + 1) * NFREE], ob[:]
                )

        # software pipeline: conv(pair k) interleaves with quant(pair k+1).
        # First input chunk's DMA goes ahead of the weight DMAs on the queue.
        slices_k = alloc_slices()
        emit_quant(0, slices_k, 0)
        nc.sync.dma_start(w_t[:], wsb[:])
        nc.sync.dma_start(sb_t[:], sb[:])
        for ch in range(1, len(CHUNKS)):
            emit_quant(0, slices_k, ch)
        for pr in range(n_pairs):
            slices_next = alloc_slices() if pr + 1 < n_pairs else None
            for s in range(N_SLICES):
                if slices_next is not None:
                    for ch in CHUNKS_FOR_SWEEP[s]:
                        emit_quant(pr + 1, slices_next, ch)
                emit_sweep(pr, slices_k, s)
            slices_k = slices_next

    _fuse_tap_ldweights(nc)
    _split_multi_syncs(nc)
    nc.finalize()
    return nc


def _host_prep(w_q, s_exp, bias, act_exp):
    """Weights: per tap a [128,128] block = 64x64 [cin,cout] duplicated 2x2
    (rows: image halves, cols: row-block halves).  Scale/bias fold."""
    w_q = np.asarray(w_q)
    w1 = np.transpose(w_q.reshape(COUT, CIN, 9), (1, 2, 0))       # [ci, t, co]
    w2 = np.concatenate([w1, w1], axis=2)                         # [ci, t, 128]
    w2 = w2.reshape(CIN, 9 * P)
    wsb = np.concatenate([w2, w2], axis=0).astype(ml_dtypes.bfloat16)

    s_exp = np.asarray(s_exp).reshape(-1).astype(np.float64)
    scale = np.exp2(float(act_exp) + s_exp)                       # [64]
    wsum = w_q.astype(np.float64).sum(axis=(1, 2, 3))             # [64]
    bias_c = np.asarray(bias).astype(np.float64) - 128.0 * wsum * scale
    col_scale = np.tile(scale, 2).astype(np.float32)
    col_bias = np.tile(bias_c, 2).astype(np.float32)
    sb = np.stack([col_scale, col_bias], axis=1)                  # [128, 2] f32
    return wsb, sb


def _decode_out(y):
    """[4, 128, 6272] bf16 -> [4, 64, 112, 112] f32.
    p = blk*64 + ch; free = it*448 + r*112 + w;
    output row = 8*it + 4*blk + r."""
    y = np.asarray(y).astype(np.float32)
    y = y.reshape(IMGS_PER_CORE, 2, CIN, N_ITERS, ROWS_PER_TILE, W)
    #            img            blk  ch   it       r             w
    y = np.transpose(y, (0, 2, 3, 1, 4, 5))      # img ch it blk r w
    return y.reshape(IMGS_PER_CORE, COUT, H, W)


def kernel(x, w_q, s_exp, bias, act_exp):
    from concourse.bass_utils import run_bass_kernel_spmd

    x = np.ascontiguousarray(np.asarray(x, dtype=np.float32))
    wsb, sb = _host_prep(np.asarray(w_q), s_exp, bias, int(act_exp))

    if "nc" not in _NC_CACHE:
        _NC_CACHE["nc"] = build_nc()
    nc = _NC_CACHE["nc"]

    in_maps = [
        {"xin": x[4 * c:4 * c + 4].reshape(2 * P, H, W), "wsb": wsb, "sb": sb}
        for c in range(N_CORES)
    ]
    _NC_CACHE["in_maps"] = in_maps
    res = run_bass_kernel_spmd(nc, in_maps, core_ids=list(range(N_CORES)))
    out = np.concatenate(
        [_decode_out(res.results[c]["yout"]) for c in range(N_CORES)], axis=0
    )
    return np.ascontiguousarray(out, dtype=np.float32)


# revision 41
# speedup vs baseline: 1.1884x; 1.0066x over previous
"""BitConv2d (ternary-weight 3x3 conv, power-of-two rescale) on 8 TRN2 NeuronCores.

Strategy:
  - Data-parallel over batch: 32 images -> 4 per core (2 image pairs).
  - Activation quantization x_int = clip(round(clip(x,-1,1)/2^-6), -127, 127)
    computed exactly: i16 = RNE(64*x + 128) on GpSimd (hw f32->i16 cast),
    bf16 = clip(i16, 64, 192) on DVE -> v = x_int + 128 (exact ints in bf16).
    Padded border cells are memset to 128 so the offset contributes exactly
    128*sum(w) per output channel, folded into the bias on the host.
  - Conv as accumulating 64x64-quadrant matmuls (K=Cin=64, M=Cout=64),
    4-per-array via tile_position quadrants:
      rows 0-63 = image (2k) channels, rows 64-127 = image (2k+1),
      cols 0-63 = psum parts 0-63  (output row-block r),
      cols 64-127 = psum parts 64-127 (row-block r+1).
    Each psum bank [128, 448] holds ONE image's two row-blocks; weight loads
    are amortized over SWEEP=2 iterations (8 matmuls per 4 quadrant loads).
  - Epilogue y = psum * 2^(act_exp+s_exp[c]) + bias'[c] in one full-width
    [128,448] ACT op per image per iter, written as bf16 into a small
    per-(sweep, image) buffer laid out so each partition's data is one
    contiguous DRAM segment (1 descriptor/partition), stored immediately.
  - The activation image is quantized into 7 slice tiles per pair (16 output
    rows + 2-row halo each); sweep s touches only slice s, so conv starts as
    soon as the first ~17 input rows are quantized.
  - I/O: 16 input DMAs (16-row chunks, [128p x 7KB]) and 28 output DMAs
    ([128p x 1.8KB] bf16); the HWDGE queue cost is mostly per-DMA, so few
    big-descriptor DMAs.  Output layout [img][p=(blk,ch)][it*448 + r*112 + w]
    with output row = 8*it + 4*blk + r, decoded on the host.
"""

import numpy as np
import ml_dtypes
from contextlib import ExitStack

_NC_CACHE = {}

N_CORES = 8
H = W = 112
HP = H + 2  # padded
CIN = COUT = 64
P = 128
IMGS_PER_CORE = 4
ROWS_PER_TILE = 4             # output rows per matmul tile (N = 4*112 = 448)
NFREE = ROWS_PER_TILE * W     # 448
N_ITERS = 14                  # 8-row output iterations per image pair
N_SLICES = 7                  # xq slices per pair; slice s <-> sweep s
SLICE_SLOTS = 18              # padded rows per slice (16 + 2-row halo)
OBUF_N = N_ITERS * NFREE      # 6272 bf16 per partition per image


def _patch_tile_drain(tile_mod):
    """This walrus build rejects a Drain carrying many sync waits; split the
    final Tile drain into single-wait sync nops."""
    from concourse.vector_clock import ScopedClock, VectorClock

    if getattr(tile_mod.TileContext, "_drain_patched", False):
        return

    def _drain_and_barrier_split(self, tick_clock, wait_clock):
        vclock = tick_clock.global_clock
        n = len(vclock)
        for proc in range(n):
            t = vclock[proc]
            if t <= 0:
                continue
            vec = [0] * n
            vec[proc] = t
            nop = self.nc.sync.nop()
            wait_clock.add_sem_waits(nop.ins, ScopedClock({None: VectorClock(vec)}))
        self.nc.sync.drain()
        assert self.sems is not None
        popped = self.nc._tile_sem_poison_stack.pop()
        assert popped is self._sem_poison
        self.nc.all_engine_barrier()
        self.nc.clear_and_free_semaphores(list(self.sems.allocated().values()))
        self.nc.all_engine_barrier()

    tile_mod.TileContext._drain_and_barrier = _drain_and_barrier_split
    tile_mod.TileContext._drain_patched = True


def _split_multi_syncs(nc):
    """This walrus build accepts at most ONE sync wait (and one update) per
    instruction.  Hoist extra waits onto preceding nops and extra updates onto
    following nops (same engine, so ordering semantics are preserved)."""
    import concourse.mybir as mybir

    fn = nc.m.functions[0]
    ctr = 0
    for bb in fn.blocks:
        new_insts = []
        for inst in bb.instructions:
            si = inst.sync_info
            pre, post = [], []
            if si is not None and si.on_wait and len(si.on_wait) > 1:
                for w in list(si.on_wait[:-1]):
                    ctr += 1
                    pre.append(
                        mybir.InstNoOp(
                            name=f"wsplit_nop_{ctr}",
                            engine=inst.engine,
                            sync_info=mybir.SyncInfo(on_wait=[w], on_update=[]),
                        )
                    )
                si.on_wait = [si.on_wait[-1]]
            if (
                si is not None
                and si.on_update
                and len(si.on_update) > 1
                and not isinstance(inst, (mybir.InstDMACopy, mybir.InstDMA))
            ):
                for u in list(si.on_update[1:]):
                    ctr += 1
                    post.append(
                        mybir.InstNoOp(
                            name=f"usplit_nop_{ctr}",
                            engine=inst.engine,
                            sync_info=mybir.SyncInfo(on_wait=[], on_update=[u]),
                        )
                    )
                si.on_update = [si.on_update[0]]
            new_insts.extend(pre)
            new_insts.append(inst)
            new_insts.extend(post)
        if len(new_insts) != len(bb.instructions):
            bb.instructions[:] = new_insts
    for bb in fn.blocks:
        for inst in bb.instructions:
            if inst.name.startswith(("wsplit_nop_", "usplit_nop_")):
                if inst.name not in nc.inst_map:
                    nc.register_instruction(inst)
    return ctr


def _fuse_tap_ldweights(nc):
    """Tile emits one InstLdweights per matmul.  Within a tap-sweep the same
    four quadrant loads repeat for every iteration; delete an InstLdweights
    when the identical load (same tile_position, tensor, offset) is already
    resident in that quadrant, and mark every matmul ldweights=False so
    walrus doesn't re-synthesize loads for the now-bare matmuls.  Sync
    waits/updates of deleted loads are preserved on PE nops."""
    import concourse.mybir as mybir

    fn = nc.m.functions[0]
    ctr = 0
    n_deleted = 0
    for bb in fn.blocks:
        new = []
        resident = {}        # tile_position -> (memref, offset)
        changed = False
        for inst in bb.instructions:
            if isinstance(inst, mybir.InstLdweights):
                pos = tuple(inst.tile_position or (0, 0))
                ap = inst.ins[0]
                key = (ap.memref, ap.offset, tuple(tuple(d) for d in ap.ap))
                if resident.get(pos) == key:
                    si = inst.sync_info
                    if si is not None:
                        for w_ in list(si.on_wait or []):
                            ctr += 1
                            new.append(mybir.InstNoOp(
                                name=f"ldwfuse_w_{ctr}", engine=inst.engine,
                                sync_info=mybir.SyncInfo(
                                    on_wait=[w_], on_update=[]),
                            ))
                        for u in list(si.on_update or []):
                            ctr += 1
                            new.append(mybir.InstNoOp(
                                name=f"ldwfuse_u_{ctr}", engine=inst.engine,
                                sync_info=mybir.SyncInfo(
                                    on_wait=[], on_update=[u]),
                            ))
                    changed = True
                    n_deleted += 1
                    continue
                resident[pos] = key
            elif isinstance(inst, mybir.InstMatmult):
                inst.ldweights = False
            elif isinstance(inst, mybir.InstDrain):
                resident.clear()
            new.append(inst)
        if changed:
            bb.instructions[:] = new
    for bb in fn.blocks:
        for inst in bb.instructions:
            if inst.name.startswith(("ldwfuse_w_", "ldwfuse_u_")):
                if inst.name not in nc.inst_map:
                    nc.register_instruction(inst)
    return n_deleted


def build_nc():
    import concourse.bass as bass
    import concourse.mybir as mybir
    import concourse.tile as tile

    _patch_tile_drain(tile)

    f32 = mybir.dt.float32
    bf16 = mybir.dt.bfloat16
    i16 = mybir.dt.int16
    Alu = mybir.AluOpType
    Act = mybir.ActivationFunctionType

    nc = bass.Bass(trn_type="TRN2")
    xin = nc.dram_tensor("xin", (2 * P, H, W), f32, kind="ExternalInput")
    wsb = nc.dram_tensor("wsb", (P, 9 * P), bf16, kind="ExternalInput")
    sb = nc.dram_tensor("sb", (P, 2), f32, kind="ExternalInput")
    yout = nc.dram_tensor(
        "yout", (IMGS_PER_CORE, P, OBUF_N), bf16, kind="ExternalOutput"
    )

    n_pairs = IMGS_PER_CORE // 2

    # quant chunk c supplies everything slice c still needs (chunk c done =>
    # slice c complete); chunk 0 is split for a faster first slice.
    CHUNKS = [(0, 7), (8, 16)] + [
        (16 * c + 1, min(16 * c + 16, H - 1)) for c in range(1, N_SLICES)
    ]
    CHUNKS_FOR_SWEEP = [[0, 1], [2], [3], [4], [5], [6], [7]]

    with tile.TileContext(nc) as tc, ExitStack() as ctx:
        const_pool = ctx.enter_context(tc.tile_pool(name="const", bufs=1))
        xq_pool = ctx.enter_context(tc.tile_pool(name="xq", bufs=2 * N_SLICES))
        stg_pool = ctx.enter_context(tc.tile_pool(name="stg", bufs=8))
        rnd_pool = ctx.enter_context(tc.tile_pool(name="rnd", bufs=8))
        out_pool = ctx.enter_context(tc.tile_pool(name="out", bufs=8))
        psum_pool = ctx.enter_context(
            tc.tile_pool(name="psum", bufs=8, space=bass.MemorySpace.PSUM)
        )

        w_t = const_pool.tile([P, 9 * P], bf16)
        sb_t = const_pool.tile([P, 2], f32)

        def alloc_slices():
            # slice s holds padded rows 16s .. 16s+17 (2-row halo between
            # consecutive slices); sweep s reads only slice s.
            sl = [xq_pool.tile([P, SLICE_SLOTS, HP], bf16, name="xq")
                  for _ in range(N_SLICES)]
            for s, t in enumerate(sl):
                nc.vector.memset(t[:, :, 0], 128.0)
                nc.vector.memset(t[:, :, HP - 1], 128.0)
            nc.vector.memset(sl[0][:, 0, :], 128.0)
            nc.vector.memset(sl[-1][:, SLICE_SLOTS - 1, :], 128.0)
            return sl

        def emit_quant(pr, slices, ch):
            r0, r1 = CHUNKS[ch]
            nrows = r1 - r0 + 1
            stg = stg_pool.tile([P, nrows, W], f32, name="stg")
            nc.sync.dma_start(stg[:], xin[pr * P:(pr + 1) * P, r0:r1 + 1, :])
            rnd = rnd_pool.tile([P, nrows, W], i16, name="rnd")
            # i16 = RNE(64*x + 128): hw f32->i16 cast rounds to nearest even
            nc.gpsimd.tensor_scalar(out=rnd[:], in0=stg[:], scalar1=64.0,
                                    scalar2=128.0, op0=Alu.mult, op1=Alu.add)
            # bf16 = clip(i16, 64, 192) == x_int + 128 into the slice tiles
            # (padded rows r0+1 .. r1+1; slice s spans 16s .. 16s+17)
            p0, p1 = r0 + 1, r1 + 1
            for s in range(N_SLICES):
                lo, hi = max(p0, 16 * s), min(p1, 16 * s + SLICE_SLOTS - 1)
                if lo > hi:
                    continue
                nc.vector.tensor_scalar(
                    out=slices[s][:, lo - 16 * s:hi - 16 * s + 1, 1:1 + W],
                    in0=rnd[:, lo - p0:hi - p0 + 1, :],
                    scalar1=64, scalar2=192, op0=Alu.max, op1=Alu.min,
                )

        def emit_sweep(pr, slices, s):
            """Conv iters 2s, 2s+1 from slice s.  Per (iter, image) one psum
            bank holds both row-blocks; redundant quadrant weight reloads are
            stripped afterwards by _fuse_tap_ldweights."""
            its = (2 * s, 2 * s + 1)
            xq = slices[s]
            ps = {}
            for it in its:
                for im in range(2):
                    ps[(it, im)] = psum_pool.tile([P, NFREE], f32, name="ps")
            for tap in range(9):
                dh, dw = divmod(tap, 3)
                first, last = tap == 0, tap == 8
                for it in its:
                    base = it * 8 - 16 * s
                    for im, r in ((0, 0), (1, 64)):
                        for blk in range(2):
                            hs = base + ROWS_PER_TILE * blk + dh
                            c = 64 * blk
                            nc.tensor.matmul(
                                ps[(it, im)][c:c + 64, :],
                                w_t[r:r + 64, tap * P + c:tap * P + c + 64],
                                xq[r:r + 64, hs:hs + ROWS_PER_TILE, dw:dw + W],
                                start=first, stop=last,
                            )
            # epilogue: one full-width op per (iter, image), bf16 out into a
            # small per-(sweep, image) buffer, stored immediately.  ACT only
            # (DVE epilogues stall the quant-clip supply chain) -- except the
            # final two sweeps, where quant is long finished and splitting
            # image b onto DVE halves the drain tail.
            drain_split = pr == n_pairs - 1 and s >= N_SLICES - 2
            for im in range(2):
                ob = out_pool.tile([P, 2 * NFREE], bf16, name="ob")
                for j, it in enumerate(its):
                    dst = ob[:, j * NFREE:(j + 1) * NFREE]
                    if im == 1 and drain_split:
                        nc.vector.tensor_scalar(
                            out=dst, in0=ps[(it, im)][:],
                            scalar1=sb_t[:, 0:1], scalar2=sb_t[:, 1:2],
                            op0=Alu.mult, op1=Alu.add,
                        )
                    else:
                        nc.scalar.activation(
                            dst, ps[(it, im)][:], Act.Identity,
                            scale=sb_t[:, 0:1], bias=sb_t[:, 1:2],
                        )
                img = 2 * pr + im
                nc.sync.dma_start(
                    yout[img, :, its[0] * NFREE:(its[-1] + 1) * NFREE], ob[:]
                )

        # software pipeline: conv(pair k) interleaves with quant(pair k+1).
        # First input chunk's DMA goes ahead of the weight DMAs on the queue.
        slices_k = alloc_slices()
        emit_quant(0, slices_k, 0)
        nc.sync.dma_start(w_t[:], wsb[:])
        nc.sync.dma_start(sb_t[:], sb[:])
        for ch in range(1, len(CHUNKS)):
            emit_quant(0, slices_k, ch)
        for pr in range(n_pairs):
            slices_next = alloc_slices() if pr + 1 < n_pairs else None
            for s in range(N_SLICES):
                if slices_next is not None:
                    for ch in CHUNKS_FOR_SWEEP[s]:
                        emit_quant(pr + 1, slices_next, ch)
                emit_sweep(pr, slices_k, s)
            slices_k = slices_next

    _fuse_tap_ldweights(nc)
    _split_multi_syncs(nc)
    nc.finalize()
    return nc


def _host_prep(w_q, s_exp, bias, act_exp):
    """Weights: per tap a [128,128] block = 64x64 [cin,cout] duplicated 2x2
    (rows: image halves, cols: row-block halves).  Scale/bias fold."""
    w_q = np.asarray(w_q)
    w1 = np.transpose(w_q.reshape(COUT, CIN, 9), (1, 2, 0))       # [ci, t, co]
    w2 = np.concatenate([w1, w1], axis=2)                         # [ci, t, 128]
    w2 = w2.reshape(CIN, 9 * P)
    wsb = np.concatenate([w2, w2], axis=0).astype(ml_dtypes.bfloat16)

    s_exp = np.asarray(s_exp).reshape(-1).astype(np.float64)
    scale = np.exp2(float(act_exp) + s_exp)                       # [64]
    wsum = w_q.astype(np.float64).sum(axis=(1, 2, 3))             # [64]
    bias_c = np.asarray(bias).astype(np.float64) - 128.0 * wsum * scale
    col_scale = np.tile(scale, 2).astype(np.float32)
    col_bias = np.tile(bias_c, 2).astype(np.float32)
    sb = np.stack([col_scale, col_bias], axis=1)                  # [128, 2] f32
    return wsb, sb


def _decode_out(y):
    """[4, 128, 6272] bf16 -> [4, 64, 112, 112] f32.
    p = blk*64 + ch; free = it*448 + r*112 + w;
    output row = 8*it + 4*blk + r."""
    y = np.asarray(y).astype(np.float32)
    y = y.reshape(IMGS_PER_CORE, 2, CIN, N_ITERS, ROWS_PER_TILE, W)
    #            img            blk  ch   it       r             w
    y = np.transpose(y, (0, 2, 3, 1, 4, 5))      # img ch it blk r w
    return y.reshape(IMGS_PER_CORE, COUT, H, W)


def kernel(x, w_q, s_exp, bias, act_exp):
    from concourse.bass_utils import run_bass_kernel_spmd

    x = np.ascontiguousarray(np.asarray(x, dtype=np.float32))
    wsb, sb = _host_prep(np.asarray(w_q), s_exp, bias, int(act_exp))

    if "nc" not in _NC_CACHE:
        _NC_CACHE["nc"] = build_nc()
    nc = _NC_CACHE["nc"]

    in_maps = [
        {"xin": x[4 * c:4 * c + 4].reshape(2 * P, H, W), "wsb": wsb, "sb": sb}
        for c in range(N_CORES)
    ]
    _NC_CACHE["in_maps"] = in_maps
    res = run_bass_kernel_spmd(nc, in_maps, core_ids=list(range(N_CORES)))
    out = np.concatenate(
        [_decode_out(res.results[c]["yout"]) for c in range(N_CORES)], axis=0
    )
    return np.ascontiguousarray(out, dtype=np.float32)


# revision 45
# speedup vs baseline: 1.3330x; 1.1216x over previous
"""BitConv2d (ternary-weight 3x3 conv, power-of-two rescale) on 8 TRN2 NeuronCores.

Strategy:
  - Data-parallel over batch: 32 images -> 4 per core (2 image pairs).
  - Activation quantization x_int = clip(round(clip(x,-1,1)/2^-6), -127, 127)
    computed exactly: i16 = RNE(64*x + 128) on GpSimd (hw f32->i16 cast),
    bf16 = clip(i16, 64, 192) on DVE -> v = x_int + 128 (exact ints in bf16).
    Padded border cells are memset to 128 so the offset contributes exactly
    128*sum(w) per output channel, folded into the bias on the host.
  - Conv as accumulating 64x64-quadrant matmuls (K=Cin=64, M=Cout=64),
    4-per-array via tile_position quadrants:
      rows 0-63 = image (2k) channels, rows 64-127 = image (2k+1),
      cols 0-63 = psum parts 0-63  (output row-block r),
      cols 64-127 = psum parts 64-127 (row-block r+1).
    Each psum bank [128, 448] holds ONE image's two row-blocks; weight loads
    are amortized over SWEEP=2 iterations (8 matmuls per 4 quadrant loads).
  - Epilogue y = psum * 2^(act_exp+s_exp[c]) + bias'[c] in one full-width
    [128,448] ACT op per image per iter, written as bf16 into a small
    per-(sweep, image) buffer laid out so each partition's data is one
    contiguous DRAM segment (1 descriptor/partition), stored immediately.
  - The activation image is quantized into 7 slice tiles per pair (16 output
    rows + 2-row halo each); sweep s touches only slice s, so conv starts as
    soon as the first ~17 input rows are quantized.
  - I/O: 16 input DMAs (16-row chunks, [128p x 7KB]) and 28 output DMAs
    ([128p x 1.8KB] bf16); the HWDGE queue cost is mostly per-DMA, so few
    big-descriptor DMAs.  Output layout [img][p=(blk,ch)][it*448 + r*112 + w]
    with output row = 8*it + 4*blk + r, decoded on the host.
"""

import numpy as np
import ml_dtypes
from contextlib import ExitStack

_NC_CACHE = {}

N_CORES = 8
H = W = 112
HP = H + 2  # padded
CIN = COUT = 64
P = 128
IMGS_PER_CORE = 4
ROWS_PER_TILE = 4             # output rows per matmul tile (N = 4*112 = 448)
NFREE = ROWS_PER_TILE * W     # 448
N_ITERS = 14                  # 8-row output iterations per image pair
N_SLICES = 7                  # xq slices per pair; slice s <-> sweep s
SLICE_SLOTS = 18              # padded rows per slice (16 + 2-row halo)
OBUF_N = N_ITERS * NFREE      # 6272 bf16 per partition per image


def _patch_tile_drain(tile_mod):
    """This walrus build rejects a Drain carrying many sync waits; split the
    final Tile drain into single-wait sync nops."""
    from concourse.vector_clock import ScopedClock, VectorClock

    if getattr(tile_mod.TileContext, "_drain_patched", False):
        return

    def _drain_and_barrier_split(self, tick_clock, wait_clock):
        vclock = tick_clock.global_clock
        n = len(vclock)
        for proc in range(n):
            t = vclock[proc]
            if t <= 0:
                continue
            vec = [0] * n
            vec[proc] = t
            nop = self.nc.sync.nop()
            wait_clock.add_sem_waits(nop.ins, ScopedClock({None: VectorClock(vec)}))
        self.nc.sync.drain()
        assert self.sems is not None
        popped = self.nc._tile_sem_poison_stack.pop()
        assert popped is self._sem_poison
        self.nc.all_engine_barrier()
        self.nc.clear_and_free_semaphores(list(self.sems.allocated().values()))
        self.nc.all_engine_barrier()

    tile_mod.TileContext._drain_and_barrier = _drain_and_barrier_split
    tile_mod.TileContext._drain_patched = True


def _split_multi_syncs(nc):
    """This walrus build accepts at most ONE sync wait (and one update) per
    instruction.  Hoist extra waits onto preceding nops and extra updates onto
    following nops (same engine, so ordering semantics are preserved)."""
    import concourse.mybir as mybir

    fn = nc.m.functions[0]
    ctr = 0
    for bb in fn.blocks:
        new_insts = []
        for inst in bb.instructions:
            si = inst.sync_info
            pre, post = [], []
            if si is not None and si.on_wait and len(si.on_wait) > 1:
                for w in list(si.on_wait[:-1]):
                    ctr += 1
                    pre.append(
                        mybir.InstNoOp(
                            name=f"wsplit_nop_{ctr}",
                            engine=inst.engine,
                            sync_info=mybir.SyncInfo(on_wait=[w], on_update=[]),
                        )
                    )
                si.on_wait = [si.on_wait[-1]]
            if (
                si is not None
                and si.on_update
                and len(si.on_update) > 1
                and not isinstance(inst, (mybir.InstDMACopy, mybir.InstDMA))
            ):
                for u in list(si.on_update[1:]):
                    ctr += 1
                    post.append(
                        mybir.InstNoOp(
                            name=f"usplit_nop_{ctr}",
                            engine=inst.engine,
                            sync_info=mybir.SyncInfo(on_wait=[], on_update=[u]),
                        )
                    )
                si.on_update = [si.on_update[0]]
            new_insts.extend(pre)
            new_insts.append(inst)
            new_insts.extend(post)
        if len(new_insts) != len(bb.instructions):
            bb.instructions[:] = new_insts
    for bb in fn.blocks:
        for inst in bb.instructions:
            if inst.name.startswith(("wsplit_nop_", "usplit_nop_")):
                if inst.name not in nc.inst_map:
                    nc.register_instruction(inst)
    return ctr


def _fuse_tap_ldweights(nc):
    """Tile emits one InstLdweights per matmul.  Within a tap-sweep the same
    four quadrant loads repeat for every iteration; delete an InstLdweights
    when the identical load (same tile_position, tensor, offset) is already
    resident in that quadrant, and mark every matmul ldweights=False so
    walrus doesn't re-synthesize loads for the now-bare matmuls.  Sync
    waits/updates of deleted loads are preserved on PE nops."""
    import concourse.mybir as mybir

    fn = nc.m.functions[0]
    ctr = 0
    n_deleted = 0
    for bb in fn.blocks:
        new = []
        resident = {}        # tile_position -> (memref, offset)
        changed = False
        for inst in bb.instructions:
            if isinstance(inst, mybir.InstLdweights):
                pos = tuple(inst.tile_position or (0, 0))
                ap = inst.ins[0]
                key = (ap.memref, ap.offset, tuple(tuple(d) for d in ap.ap))
                if resident.get(pos) == key:
                    si = inst.sync_info
                    if si is not None:
                        for w_ in list(si.on_wait or []):
                            ctr += 1
                            new.append(mybir.InstNoOp(
                                name=f"ldwfuse_w_{ctr}", engine=inst.engine,
                                sync_info=mybir.SyncInfo(
                                    on_wait=[w_], on_update=[]),
                            ))
                        for u in list(si.on_update or []):
                            ctr += 1
                            new.append(mybir.InstNoOp(
                                name=f"ldwfuse_u_{ctr}", engine=inst.engine,
                                sync_info=mybir.SyncInfo(
                                    on_wait=[], on_update=[u]),
                            ))
                    changed = True
                    n_deleted += 1
                    continue
                resident[pos] = key
            elif isinstance(inst, mybir.InstMatmult):
                inst.ldweights = False
            elif isinstance(inst, mybir.InstDrain):
                resident.clear()
            new.append(inst)
        if changed:
            bb.instructions[:] = new
    for bb in fn.blocks:
        for inst in bb.instructions:
            if inst.name.startswith(("ldwfuse_w_", "ldwfuse_u_")):
                if inst.name not in nc.inst_map:
                    nc.register_instruction(inst)
    return n_deleted


def build_nc():
    import concourse.bass as bass
    import concourse.mybir as mybir
    import concourse.tile as tile

    _patch_tile_drain(tile)

    f32 = mybir.dt.float32
    bf16 = mybir.dt.bfloat16
    i16 = mybir.dt.int16
    Alu = mybir.AluOpType
    Act = mybir.ActivationFunctionType

    nc = bass.Bass(trn_type="TRN2")
    xin = nc.dram_tensor("xin", (2 * P, H, W), f32, kind="ExternalInput")
    wsb = nc.dram_tensor("wsb", (P, 9 * P), bf16, kind="ExternalInput")
    sb = nc.dram_tensor("sb", (P, 2), f32, kind="ExternalInput")
    yout = nc.dram_tensor(
        "yout", (IMGS_PER_CORE, P, OBUF_N), bf16, kind="ExternalOutput"
    )

    n_pairs = IMGS_PER_CORE // 2

    # quant chunk c supplies everything slice c still needs (chunk c done =>
    # slice c complete); chunk 0 is split for a faster first slice.
    CHUNKS = [(0, 7), (8, 16)] + [
        (16 * c + 1, min(16 * c + 16, H - 1)) for c in range(1, N_SLICES)
    ]
    CHUNKS_FOR_SWEEP = [[0, 1], [2], [3], [4], [5], [6], [7]]

    with tile.TileContext(nc) as tc, ExitStack() as ctx:
        const_pool = ctx.enter_context(tc.tile_pool(name="const", bufs=1))
        xq_pool = ctx.enter_context(tc.tile_pool(name="xq", bufs=2 * N_SLICES))
        stg_pool = ctx.enter_context(tc.tile_pool(name="stg", bufs=8))
        rnd_pool = ctx.enter_context(tc.tile_pool(name="rnd", bufs=8))
        out_pool = ctx.enter_context(tc.tile_pool(name="out", bufs=8))
        obig_pool = ctx.enter_context(tc.tile_pool(name="obig", bufs=2))
        psum_pool = ctx.enter_context(
            tc.tile_pool(name="psum", bufs=8, space=bass.MemorySpace.PSUM)
        )

        w_t = const_pool.tile([P, 9 * P], bf16)
        sb_t = const_pool.tile([P, 2], f32)

        def alloc_slices():
            # slice s holds padded rows 16s .. 16s+17 (2-row halo between
            # consecutive slices); sweep s reads only slice s.
            sl = [xq_pool.tile([P, SLICE_SLOTS, HP], bf16, name="xq")
                  for _ in range(N_SLICES)]
            for s, t in enumerate(sl):
                nc.vector.memset(t[:, :, 0], 128.0)
                nc.vector.memset(t[:, :, HP - 1], 128.0)
            nc.vector.memset(sl[0][:, 0, :], 128.0)
            nc.vector.memset(sl[-1][:, SLICE_SLOTS - 1, :], 128.0)
            return sl

        def emit_quant(pr, slices, ch):
            r0, r1 = CHUNKS[ch]
            nrows = r1 - r0 + 1
            stg = stg_pool.tile([P, nrows, W], f32, name="stg")
            nc.sync.dma_start(stg[:], xin[pr * P:(pr + 1) * P, r0:r1 + 1, :])
            rnd = rnd_pool.tile([P, nrows, W], i16, name="rnd")
            # i16 = RNE(64*x + 128): hw f32->i16 cast rounds to nearest even
            nc.gpsimd.tensor_scalar(out=rnd[:], in0=stg[:], scalar1=64.0,
                                    scalar2=128.0, op0=Alu.mult, op1=Alu.add)
            # bf16 = clip(i16, 64, 192) == x_int + 128 into the slice tiles
            # (padded rows r0+1 .. r1+1; slice s spans 16s .. 16s+17)
            p0, p1 = r0 + 1, r1 + 1
            for s in range(N_SLICES):
                lo, hi = max(p0, 16 * s), min(p1, 16 * s + SLICE_SLOTS - 1)
                if lo > hi:
                    continue
                nc.vector.tensor_scalar(
                    out=slices[s][:, lo - 16 * s:hi - 16 * s + 1, 1:1 + W],
                    in0=rnd[:, lo - p0:hi - p0 + 1, :],
                    scalar1=64, scalar2=192, op0=Alu.max, op1=Alu.min,
                )

        def emit_sweep(pr, slices, s, obig=None):
            """Conv iters 2s, 2s+1 from slice s.  Per (iter, image) one psum
            bank holds both row-blocks; redundant quadrant weight reloads are
            stripped afterwards by _fuse_tap_ldweights."""
            its = (2 * s, 2 * s + 1)
            xq = slices[s]
            ps = {}
            for it in its:
                for im in range(2):
                    ps[(it, im)] = psum_pool.tile([P, NFREE], f32, name="ps")
            for tap in range(9):
                dh, dw = divmod(tap, 3)
                first, last = tap == 0, tap == 8
                for it in its:
                    base = it * 8 - 16 * s
                    for im, r in ((0, 0), (1, 64)):
                        for blk in range(2):
                            hs = base + ROWS_PER_TILE * blk + dh
                            c = 64 * blk
                            nc.tensor.matmul(
                                ps[(it, im)][c:c + 64, :],
                                w_t[r:r + 64, tap * P + c:tap * P + c + 64],
                                xq[r:r + 64, hs:hs + ROWS_PER_TILE, dw:dw + W],
                                start=first, stop=last,
                            )
            # epilogue: one full-width op per (iter, image), ACT only (DVE
            # epilogues stall the quant-clip supply chain) -- except the
            # final two sweeps, where quant is long finished and splitting
            # image b onto DVE halves the drain tail.
            # Pair 0 accumulates into whole-image buffers flushed at its end
            # (keeps the HBM input-only while slices are being consumed);
            # pair 1 stores per-sweep so the tail stays short.
            drain_split = pr == n_pairs - 1 and s >= N_SLICES - 2
            for im in range(2):
                if obig is not None:
                    ob, off = obig[im], its[0] * NFREE
                else:
                    ob, off = out_pool.tile([P, 2 * NFREE], bf16, name="ob"), 0
                for j, it in enumerate(its):
                    dst = ob[:, off + j * NFREE:off + (j + 1) * NFREE]
                    if im == 1 and drain_split:
                        nc.vector.tensor_scalar(
                            out=dst, in0=ps[(it, im)][:],
                            scalar1=sb_t[:, 0:1], scalar2=sb_t[:, 1:2],
                            op0=Alu.mult, op1=Alu.add,
                        )
                    else:
                        nc.scalar.activation(
                            dst, ps[(it, im)][:], Act.Identity,
                            scale=sb_t[:, 0:1], bias=sb_t[:, 1:2],
                        )
                img = 2 * pr + im
                if obig is None:
                    nc.sync.dma_start(
                        yout[img, :, its[0] * NFREE:(its[-1] + 1) * NFREE],
                        ob[:],
                    )
                elif s == N_SLICES - 1:
                    nc.sync.dma_start(yout[img, :, :], ob[:])

        # software pipeline: conv(pair k) interleaves with quant(pair k+1).
        # First input chunk's DMA goes ahead of the weight DMAs on the queue.
        slices_k = alloc_slices()
        emit_quant(0, slices_k, 0)
        nc.sync.dma_start(w_t[:], wsb[:])
        nc.sync.dma_start(sb_t[:], sb[:])
        for ch in range(1, len(CHUNKS)):
            emit_quant(0, slices_k, ch)
        for pr in range(n_pairs):
            slices_next = alloc_slices() if pr + 1 < n_pairs else None
            obig = None
            if pr == 0:
                obig = [obig_pool.tile([P, OBUF_N], bf16, name="obig")
                        for _ in range(2)]
            for s in range(N_SLICES):
                if slices_next is not None:
                    for ch in CHUNKS_FOR_SWEEP[s]:
                        emit_quant(pr + 1, slices_next, ch)
                emit_sweep(pr, slices_k, s, obig)
            slices_k = slices_next

    _fuse_tap_ldweights(nc)
    _split_multi_syncs(nc)
    nc.finalize()
    return nc


def _host_prep(w_q, s_exp, bias, act_exp):
    """Weights: per tap a [128,128] block = 64x64 [cin,cout] duplicated 2x2
    (rows: image halves, cols: row-block halves).  Scale/bias fold."""
    w_q = np.asarray(w_q)
    w1 = np.transpose(w_q.reshape(COUT, CIN, 9), (1, 2, 0))       # [ci, t, co]
    w2 = np.concatenate([w1, w1], axis=2)                         # [ci, t, 128]
    w2 = w2.reshape(CIN, 9 * P)
    wsb = np.concatenate([w2, w2], axis=0).astype(ml_dtypes.bfloat16)

    s_exp = np.asarray(s_exp).reshape(-1).astype(np.float64)
    scale = np.exp2(float(act_exp) + s_exp)                       # [64]
    wsum = w_q.astype(np.float64).sum(axis=(1, 2, 3))             # [64]
    bias_c = np.asarray(bias).astype(np.float64) - 128.0 * wsum * scale
    col_scale = np.tile(scale, 2).astype(np.float32)
    col_bias = np.tile(bias_c, 2).astype(np.float32)
    sb = np.stack([col_scale, col_bias], axis=1)                  # [128, 2] f32
    return wsb, sb


def _decode_out(y):
    """[4, 128, 6272] bf16 -> [4, 64, 112, 112] f32.
    p = blk*64 + ch; free = it*448 + r*112 + w;
    output row = 8*it + 4*blk + r."""
    y = np.asarray(y).astype(np.float32)
    y = y.reshape(IMGS_PER_CORE, 2, CIN, N_ITERS, ROWS_PER_TILE, W)
    #            img            blk  ch   it       r             w
    y = np.transpose(y, (0, 2, 3, 1, 4, 5))      # img ch it blk r w
    return y.reshape(IMGS_PER_CORE, COUT, H, W)


def kernel(x, w_q, s_exp, bias, act_exp):
    from concourse.bass_utils import run_bass_kernel_spmd

    x = np.ascontiguousarray(np.asarray(x, dtype=np.float32))
    wsb, sb = _host_prep(np.asarray(w_q), s_exp, bias, int(act_exp))

    if "nc" not in _NC_CACHE:
        _NC_CACHE["nc"] = build_nc()
    nc = _NC_CACHE["nc"]

    in_maps = [
        {"xin": x[4 * c:4 * c + 4].reshape(2 * P, H, W), "wsb": wsb, "sb": sb}
        for c in range(N_CORES)
    ]
    _NC_CACHE["in_maps"] = in_maps
    res = run_bass_kernel_spmd(nc, in_maps, core_ids=list(range(N_CORES)))
    out = np.concatenate(
        [_decode_out(res.results[c]["yout"]) for c in range(N_CORES)], axis=0
    )
    return np.ascontiguousarray(out, dtype=np.float32)


# revision 49
# speedup vs baseline: 1.3543x; 1.0160x over previous
"""BitConv2d (ternary-weight 3x3 conv, power-of-two rescale) on 8 TRN2 NeuronCores.

Strategy:
  - Data-parallel over batch: 32 images -> 4 per core (2 image pairs).
  - Activation quantization x_int = clip(round(clip(x,-1,1)/2^-6), -127, 127)
    computed exactly: i16 = RNE(64*x + 128) on GpSimd (hw f32->i16 cast),
    bf16 = clip(i16, 64, 192) on DVE -> v = x_int + 128 (exact ints in bf16).
    Padded border cells are memset to 128 so the offset contributes exactly
    128*sum(w) per output channel, folded into the bias on the host.
  - Conv as accumulating 64x64-quadrant matmuls (K=Cin=64, M=Cout=64),
    4-per-array via tile_position quadrants:
      rows 0-63 = image (2k) channels, rows 64-127 = image (2k+1),
      cols 0-63 = psum parts 0-63  (output row-block r),
      cols 64-127 = psum parts 64-127 (row-block r+1).
    Each psum bank [128, 448] holds ONE image's two row-blocks; weight loads
    are amortized over SWEEP=2 iterations (8 matmuls per 4 quadrant loads).
  - Epilogue y = psum * 2^(act_exp+s_exp[c]) + bias'[c] in one full-width
    [128,448] ACT op per image per iter, written as bf16 into a small
    per-(sweep, image) buffer laid out so each partition's data is one
    contiguous DRAM segment (1 descriptor/partition), stored immediately.
  - The activation image is quantized into 7 slice tiles per pair (16 output
    rows + 2-row halo each); sweep s touches only slice s, so conv starts as
    soon as the first ~17 input rows are quantized.
  - I/O: 16 input DMAs (16-row chunks, [128p x 7KB]) and 28 output DMAs
    ([128p x 1.8KB] bf16); the HWDGE queue cost is mostly per-DMA, so few
    big-descriptor DMAs.  Output layout [img][p=(blk,ch)][it*448 + r*112 + w]
    with output row = 8*it + 4*blk + r, decoded on the host.
"""

import numpy as np
import ml_dtypes
from contextlib import ExitStack

_NC_CACHE = {}

N_CORES = 8
H = W = 112
HP = H + 2  # padded
CIN = COUT = 64
P = 128
IMGS_PER_CORE = 4
ROWS_PER_TILE = 4             # output rows per matmul tile (N = 4*112 = 448)
NFREE = ROWS_PER_TILE * W     # 448
N_ITERS = 14                  # 8-row output iterations per image pair
N_SLICES = 7                  # xq slices per pair; slice s <-> sweep s
SLICE_SLOTS = 18              # padded rows per slice (16 + 2-row halo)
OBUF_N = N_ITERS * NFREE      # 6272 bf16 per partition per image


def _patch_tile_drain(tile_mod):
    """This walrus build rejects a Drain carrying many sync waits; split the
    final Tile drain into single-wait sync nops."""
    from concourse.vector_clock import ScopedClock, VectorClock

    if getattr(tile_mod.TileContext, "_drain_patched", False):
        return

    def _drain_and_barrier_split(self, tick_clock, wait_clock):
        vclock = tick_clock.global_clock
        n = len(vclock)
        for proc in range(n):
            t = vclock[proc]
            if t <= 0:
                continue
            vec = [0] * n
            vec[proc] = t
            nop = self.nc.sync.nop()
            wait_clock.add_sem_waits(nop.ins, ScopedClock({None: VectorClock(vec)}))
        self.nc.sync.drain()
        assert self.sems is not None
        popped = self.nc._tile_sem_poison_stack.pop()
        assert popped is self._sem_poison
        self.nc.all_engine_barrier()
        self.nc.clear_and_free_semaphores(list(self.sems.allocated().values()))
        self.nc.all_engine_barrier()

    tile_mod.TileContext._drain_and_barrier = _drain_and_barrier_split
    tile_mod.TileContext._drain_patched = True


def _split_multi_syncs(nc):
    """This walrus build accepts at most ONE sync wait (and one update) per
    instruction.  Hoist extra waits onto preceding nops and extra updates onto
    following nops (same engine, so ordering semantics are preserved)."""
    import concourse.mybir as mybir

    fn = nc.m.functions[0]
    ctr = 0
    for bb in fn.blocks:
        new_insts = []
        for inst in bb.instructions:
            si = inst.sync_info
            pre, post = [], []
            if si is not None and si.on_wait and len(si.on_wait) > 1:
                for w in list(si.on_wait[:-1]):
                    ctr += 1
                    pre.append(
                        mybir.InstNoOp(
                            name=f"wsplit_nop_{ctr}",
                            engine=inst.engine,
                            sync_info=mybir.SyncInfo(on_wait=[w], on_update=[]),
                        )
                    )
                si.on_wait = [si.on_wait[-1]]
            if (
                si is not None
                and si.on_update
                and len(si.on_update) > 1
                and not isinstance(inst, (mybir.InstDMACopy, mybir.InstDMA))
            ):
                for u in list(si.on_update[1:]):
                    ctr += 1
                    post.append(
                        mybir.InstNoOp(
                            name=f"usplit_nop_{ctr}",
                            engine=inst.engine,
                            sync_info=mybir.SyncInfo(on_wait=[], on_update=[u]),
                        )
                    )
                si.on_update = [si.on_update[0]]
            new_insts.extend(pre)
            new_insts.append(inst)
            new_insts.extend(post)
        if len(new_insts) != len(bb.instructions):
            bb.instructions[:] = new_insts
    for bb in fn.blocks:
        for inst in bb.instructions:
            if inst.name.startswith(("wsplit_nop_", "usplit_nop_")):
                if inst.name not in nc.inst_map:
                    nc.register_instruction(inst)
    return ctr


def _fuse_tap_ldweights(nc):
    """Tile emits one InstLdweights per matmul.  Within a tap-sweep the same
    four quadrant loads repeat for every iteration; delete an InstLdweights
    when the identical load (same tile_position, tensor, offset) is already
    resident in that quadrant, and mark every matmul ldweights=False so
    walrus doesn't re-synthesize loads for the now-bare matmuls.  Sync
    waits/updates of deleted loads are preserved on PE nops."""
    import concourse.mybir as mybir

    fn = nc.m.functions[0]
    ctr = 0
    n_deleted = 0
    for bb in fn.blocks:
        new = []
        resident = {}        # tile_position -> (memref, offset)
        changed = False
        for inst in bb.instructions:
            if isinstance(inst, mybir.InstLdweights):
                pos = tuple(inst.tile_position or (0, 0))
                ap = inst.ins[0]
                key = (ap.memref, ap.offset, tuple(tuple(d) for d in ap.ap))
                if resident.get(pos) == key:
                    si = inst.sync_info
                    if si is not None:
                        for w_ in list(si.on_wait or []):
                            ctr += 1
                            new.append(mybir.InstNoOp(
                                name=f"ldwfuse_w_{ctr}", engine=inst.engine,
                                sync_info=mybir.SyncInfo(
                                    on_wait=[w_], on_update=[]),
                            ))
                        for u in list(si.on_update or []):
                            ctr += 1
                            new.append(mybir.InstNoOp(
                                name=f"ldwfuse_u_{ctr}", engine=inst.engine,
                                sync_info=mybir.SyncInfo(
                                    on_wait=[], on_update=[u]),
                            ))
                    changed = True
                    n_deleted += 1
                    continue
                resident[pos] = key
            elif isinstance(inst, mybir.InstMatmult):
                inst.ldweights = False
            elif isinstance(inst, mybir.InstDrain):
                resident.clear()
            new.append(inst)
        if changed:
            bb.instructions[:] = new
    for bb in fn.blocks:
        for inst in bb.instructions:
            if inst.name.startswith(("ldwfuse_w_", "ldwfuse_u_")):
                if inst.name not in nc.inst_map:
                    nc.register_instruction(inst)
    return n_deleted


def build_nc():
    import concourse.bass as bass
    import concourse.mybir as mybir
    import concourse.tile as tile

    _patch_tile_drain(tile)

    f32 = mybir.dt.float32
    bf16 = mybir.dt.bfloat16
    i16 = mybir.dt.int16
    Alu = mybir.AluOpType
    Act = mybir.ActivationFunctionType

    nc = bass.Bass(trn_type="TRN2")
    xin = nc.dram_tensor("xin", (2 * P, H, W), f32, kind="ExternalInput")
    wsb = nc.dram_tensor("wsb", (P, 9 * P), bf16, kind="ExternalInput")
    sb = nc.dram_tensor("sb", (P, 2), f32, kind="ExternalInput")
    yout = nc.dram_tensor(
        "yout", (IMGS_PER_CORE, P, OBUF_N), bf16, kind="ExternalOutput"
    )

    n_pairs = IMGS_PER_CORE // 2

    # quant chunk c supplies everything slice c still needs (chunk c done =>
    # slice c complete); chunk 0 is split for a faster first slice.
    CHUNKS = [(0, 7), (8, 16)] + [
        (16 * c + 1, min(16 * c + 16, H - 1)) for c in range(1, N_SLICES)
    ]
    CHUNKS_FOR_SWEEP = [[0, 1], [2], [3], [4], [5], [6], [7]]

    with tile.TileContext(nc) as tc, ExitStack() as ctx:
        const_pool = ctx.enter_context(tc.tile_pool(name="const", bufs=1))
        xq_pool = ctx.enter_context(tc.tile_pool(name="xq", bufs=2 * N_SLICES))
        stg_pool = ctx.enter_context(tc.tile_pool(name="stg", bufs=8))
        rnd_pool = ctx.enter_context(tc.tile_pool(name="rnd", bufs=8))
        out_pool = ctx.enter_context(tc.tile_pool(name="out", bufs=8))
        obig_pool = ctx.enter_context(tc.tile_pool(name="obig", bufs=2))
        psum_pool = ctx.enter_context(
            tc.tile_pool(name="psum", bufs=8, space=bass.MemorySpace.PSUM)
        )

        w_t = const_pool.tile([P, 9 * P], bf16)
        sb_t = const_pool.tile([P, 2], f32)

        def alloc_slices():
            # slice s holds padded rows 16s .. 16s+17 (2-row halo between
            # consecutive slices); sweep s reads only slice s.
            sl = [xq_pool.tile([P, SLICE_SLOTS, HP], bf16, name="xq")
                  for _ in range(N_SLICES)]
            for s, t in enumerate(sl):
                nc.vector.memset(t[:, :, 0], 128.0)
                nc.vector.memset(t[:, :, HP - 1], 128.0)
            nc.vector.memset(sl[0][:, 0, :], 128.0)
            nc.vector.memset(sl[-1][:, SLICE_SLOTS - 1, :], 128.0)
            return sl

        def emit_quant(pr, slices, ch):
            r0, r1 = CHUNKS[ch]
            nrows = r1 - r0 + 1
            stg = stg_pool.tile([P, nrows, W], f32, name="stg")
            nc.sync.dma_start(stg[:], xin[pr * P:(pr + 1) * P, r0:r1 + 1, :])
            rnd = rnd_pool.tile([P, nrows, W], i16, name="rnd")
            # i16 = RNE(64*x + 128): hw f32->i16 cast rounds to nearest even
            nc.gpsimd.tensor_scalar(out=rnd[:], in0=stg[:], scalar1=64.0,
                                    scalar2=128.0, op0=Alu.mult, op1=Alu.add)
            # bf16 = clip(i16, 64, 192) == x_int + 128 into the slice tiles
            # (padded rows r0+1 .. r1+1; slice s spans 16s .. 16s+17)
            p0, p1 = r0 + 1, r1 + 1
            for s in range(N_SLICES):
                lo, hi = max(p0, 16 * s), min(p1, 16 * s + SLICE_SLOTS - 1)
                if lo > hi:
                    continue
                nc.vector.tensor_scalar(
                    out=slices[s][:, lo - 16 * s:hi - 16 * s + 1, 1:1 + W],
                    in0=rnd[:, lo - p0:hi - p0 + 1, :],
                    scalar1=64, scalar2=192, op0=Alu.max, op1=Alu.min,
                )

        def emit_sweep(pr, slices, s, obig=None):
            """Conv iters 2s, 2s+1 from slice s.  Per (iter, image) one psum
            bank holds both row-blocks; redundant quadrant weight reloads are
            stripped afterwards by _fuse_tap_ldweights."""
            its = (2 * s, 2 * s + 1)
            xq = slices[s]
            ps = {}
            for it in its:
                for im in range(2):
                    ps[(it, im)] = psum_pool.tile([P, NFREE], f32, name="ps")
            for tap in range(9):
                dh, dw = divmod(tap, 3)
                first, last = tap == 0, tap == 8
                for it in its:
                    base = it * 8 - 16 * s
                    for im, r in ((0, 0), (1, 64)):
                        for blk in range(2):
                            hs = base + ROWS_PER_TILE * blk + dh
                            c = 64 * blk
                            nc.tensor.matmul(
                                ps[(it, im)][c:c + 64, :],
                                w_t[r:r + 64, tap * P + c:tap * P + c + 64],
                                xq[r:r + 64, hs:hs + ROWS_PER_TILE, dw:dw + W],
                                start=first, stop=last,
                            )
            # epilogue: one full-width op per (iter, image), ACT only (DVE
            # epilogues stall the quant-clip supply chain) -- except the
            # final two sweeps, where quant is long finished and splitting
            # image b onto DVE halves the drain tail.
            # Pair 0 accumulates into whole-image buffers flushed at its end
            # (keeps the HBM input-only while slices are being consumed);
            # pair 1 stores per-sweep so the tail stays short.
            if pr == n_pairs - 1 and s == N_SLICES - 1:
                # very last sweep: per-iteration stores, it-major, so the
                # iter-12 drain fully overlaps iter-13's matmuls.
                for j, it in enumerate(its):
                    for im in range(2):
                        obt = out_pool.tile([P, NFREE], bf16, name="obt")
                        if im == 1:
                            nc.vector.tensor_scalar(
                                out=obt[:], in0=ps[(it, im)][:],
                                scalar1=sb_t[:, 0:1], scalar2=sb_t[:, 1:2],
                                op0=Alu.mult, op1=Alu.add,
                            )
                        else:
                            nc.scalar.activation(
                                obt[:], ps[(it, im)][:], Act.Identity,
                                scale=sb_t[:, 0:1], bias=sb_t[:, 1:2],
                            )
                        nc.sync.dma_start(
                            yout[2 * pr + im, :,
                                 it * NFREE:(it + 1) * NFREE], obt[:]
                        )
                return
            drain_split = pr == n_pairs - 1 and s >= N_SLICES - 2
            for im in range(2):
                if obig is not None:
                    ob, off = obig[im], its[0] * NFREE
                else:
                    ob, off = out_pool.tile([P, 2 * NFREE], bf16, name="ob"), 0
                for j, it in enumerate(its):
                    dst = ob[:, off + j * NFREE:off + (j + 1) * NFREE]
                    if im == 1 and drain_split:
                        nc.vector.tensor_scalar(
                            out=dst, in0=ps[(it, im)][:],
                            scalar1=sb_t[:, 0:1], scalar2=sb_t[:, 1:2],
                            op0=Alu.mult, op1=Alu.add,
                        )
                    else:
                        nc.scalar.activation(
                            dst, ps[(it, im)][:], Act.Identity,
                            scale=sb_t[:, 0:1], bias=sb_t[:, 1:2],
                        )
                img = 2 * pr + im
                if obig is None:
                    nc.sync.dma_start(
                        yout[img, :, its[0] * NFREE:(its[-1] + 1) * NFREE],
                        ob[:],
                    )
                elif s == N_SLICES - 1:
                    nc.sync.dma_start(yout[img, :, :], ob[:])

        # software pipeline: conv(pair k) interleaves with quant(pair k+1).
        # First input chunk's DMA goes ahead of the weight DMAs on the queue.
        slices_k = alloc_slices()
        emit_quant(0, slices_k, 0)
        nc.sync.dma_start(w_t[:], wsb[:])
        nc.sync.dma_start(sb_t[:], sb[:])
        # PE warm-up: ~40 dummy N=128 matmuls run during the initial
        # DMA/quant fill so the HAM clock gate is at 2.4 GHz (not the cold
        # 1.2 GHz) when the first real matmuls issue.
        wdum = const_pool.tile([P, P], bf16, name="wdum")
        nc.vector.memset(wdum[:], 0.0)
        psd = psum_pool.tile([P, NFREE], f32, name="ps")
        for _ in range(40):
            nc.tensor.matmul(psd[:, 0:P], wdum[:], wdum[:, 0:P],
                             start=True, stop=True)
        for ch in range(1, len(CHUNKS)):
            emit_quant(0, slices_k, ch)
        for pr in range(n_pairs):
            slices_next = alloc_slices() if pr + 1 < n_pairs else None
            obig = None
            if pr == 0:
                obig = [obig_pool.tile([P, OBUF_N], bf16, name="obig")
                        for _ in range(2)]
            for s in range(N_SLICES):
                if slices_next is not None:
                    for ch in CHUNKS_FOR_SWEEP[s]:
                        emit_quant(pr + 1, slices_next, ch)
                emit_sweep(pr, slices_k, s, obig)
            slices_k = slices_next

    _fuse_tap_ldweights(nc)
    _split_multi_syncs(nc)
    nc.finalize()
    return nc


def _host_prep(w_q, s_exp, bias, act_exp):
    """Weights: per tap a [128,128] block = 64x64 [cin,cout] duplicated 2x2
    (rows: image halves, cols: row-block halves).  Scale/bias fold."""
    w_q = np.asarray(w_q)
    w1 = np.transpose(w_q.reshape(COUT, CIN, 9), (1, 2, 0))       # [ci, t, co]
    w2 = np.concatenate([w1, w1], axis=2)                         # [ci, t, 128]
    w2 = w2.reshape(CIN, 9 * P)
    wsb = np.concatenate([w2, w2], axis=0).astype(ml_dtypes.bfloat16)

    s_exp = np.asarray(s_exp).reshape(-1).astype(np.float64)
    scale = np.exp2(float(act_exp) + s_exp)                       # [64]
    wsum = w_q.astype(np.float64).sum(axis=(1, 2, 3))             # [64]
    bias_c = np.asarray(bias).astype(np.float64) - 128.0 * wsum * scale
    col_scale = np.tile(scale, 2).astype(np.float32)
    col_bias = np.tile(bias_c, 2).astype(np.float32)
    sb = np.stack([col_scale, col_bias], axis=1)                  # [128, 2] f32
    return wsb, sb


def _decode_out(y):
    """[4, 128, 6272] bf16 -> [4, 64, 112, 112] f32.
    p = blk*64 + ch; free = it*448 + r*112 + w;
    output row = 8*it + 4*blk + r."""
    y = np.asarray(y).astype(np.float32)
    y = y.reshape(IMGS_PER_CORE, 2, CIN, N_ITERS, ROWS_PER_TILE, W)
    #            img            blk  ch   it       r             w
    y = np.transpose(y, (0, 2, 3, 1, 4, 5))      # img ch it blk r w
    return y.reshape(IMGS_PER_CORE, COUT, H, W)


def kernel(x, w_q, s_exp, bias, act_exp):
    from concourse.bass_utils import run_bass_kernel_spmd

    x = np.ascontiguousarray(np.asarray(x, dtype=np.float32))
    wsb, sb = _host_prep(np.asarray(w_q), s_exp, bias, int(act_exp))

    if "nc" not in _NC_CACHE:
        _NC_CACHE["nc"] = build_nc()
    nc = _NC_CACHE["nc"]

    in_maps = [
        {"xin": x[4 * c:4 * c + 4].reshape(2 * P, H, W), "wsb": wsb, "sb": sb}
        for c in range(N_CORES)
    ]
    _NC_CACHE["in_maps"] = in_maps
    res = run_bass_kernel_spmd(nc, in_maps, core_ids=list(range(N_CORES)))
    out = np.concatenate(
        [_decode_out(res.results[c]["yout"]) for c in range(N_CORES)], axis=0
    )
    return np.ascontiguousarray(out, dtype=np.float32)


# revision 51
# speedup vs baseline: 1.3648x; 1.0078x over previous
"""BitConv2d (ternary-weight 3x3 conv, power-of-two rescale) on 8 TRN2 NeuronCores.

Strategy:
  - Data-parallel over batch: 32 images -> 4 per core (2 image pairs).
  - Activation quantization x_int = clip(round(clip(x,-1,1)/2^-6), -127, 127)
    computed exactly: i16 = RNE(64*x + 128) on GpSimd (hw f32->i16 cast),
    bf16 = clip(i16, 64, 192) on DVE -> v = x_int + 128 (exact ints in bf16).
    Padded border cells are memset to 128 so the offset contributes exactly
    128*sum(w) per output channel, folded into the bias on the host.
  - Conv as accumulating 64x64-quadrant matmuls (K=Cin=64, M=Cout=64),
    4-per-array via tile_position quadrants:
      rows 0-63 = image (2k) channels, rows 64-127 = image (2k+1),
      cols 0-63 = psum parts 0-63  (output row-block r),
      cols 64-127 = psum parts 64-127 (row-block r+1).
    Each psum bank [128, 448] holds ONE image's two row-blocks; weight loads
    are amortized over SWEEP=2 iterations (8 matmuls per 4 quadrant loads).
  - Epilogue y = psum * 2^(act_exp+s_exp[c]) + bias'[c] in one full-width
    [128,448] ACT op per image per iter, written as bf16 into a small
    per-(sweep, image) buffer laid out so each partition's data is one
    contiguous DRAM segment (1 descriptor/partition), stored immediately.
  - The activation image is quantized into 7 slice tiles per pair (16 output
    rows + 2-row halo each); sweep s touches only slice s, so conv starts as
    soon as the first ~17 input rows are quantized.
  - I/O: 16 input DMAs (16-row chunks, [128p x 7KB]) and 28 output DMAs
    ([128p x 1.8KB] bf16); the HWDGE queue cost is mostly per-DMA, so few
    big-descriptor DMAs.  Output layout [img][p=(blk,ch)][it*448 + r*112 + w]
    with output row = 8*it + 4*blk + r, decoded on the host.
"""

import numpy as np
import ml_dtypes
from contextlib import ExitStack

_NC_CACHE = {}

N_CORES = 8
H = W = 112
HP = H + 2  # padded
CIN = COUT = 64
P = 128
IMGS_PER_CORE = 4
ROWS_PER_TILE = 4             # output rows per matmul tile (N = 4*112 = 448)
NFREE = ROWS_PER_TILE * W     # 448
N_ITERS = 14                  # 8-row output iterations per image pair
N_SLICES = 7                  # xq slices per pair; slice s <-> sweep s
SLICE_SLOTS = 18              # padded rows per slice (16 + 2-row halo)
OBUF_N = N_ITERS * NFREE      # 6272 bf16 per partition per image


def _patch_tile_drain(tile_mod):
    """This walrus build rejects a Drain carrying many sync waits; split the
    final Tile drain into single-wait sync nops."""
    from concourse.vector_clock import ScopedClock, VectorClock

    if getattr(tile_mod.TileContext, "_drain_patched", False):
        return

    def _drain_and_barrier_split(self, tick_clock, wait_clock):
        vclock = tick_clock.global_clock
        n = len(vclock)
        for proc in range(n):
            t = vclock[proc]
            if t <= 0:
                continue
            vec = [0] * n
            vec[proc] = t
            nop = self.nc.sync.nop()
            wait_clock.add_sem_waits(nop.ins, ScopedClock({None: VectorClock(vec)}))
        self.nc.sync.drain()
        assert self.sems is not None
        popped = self.nc._tile_sem_poison_stack.pop()
        assert popped is self._sem_poison
        self.nc.all_engine_barrier()
        self.nc.clear_and_free_semaphores(list(self.sems.allocated().values()))
        self.nc.all_engine_barrier()

    tile_mod.TileContext._drain_and_barrier = _drain_and_barrier_split
    tile_mod.TileContext._drain_patched = True


def _split_multi_syncs(nc):
    """This walrus build accepts at most ONE sync wait (and one update) per
    instruction.  Hoist extra waits onto preceding nops and extra updates onto
    following nops (same engine, so ordering semantics are preserved)."""
    import concourse.mybir as mybir

    fn = nc.m.functions[0]
    ctr = 0
    for bb in fn.blocks:
        new_insts = []
        for inst in bb.instructions:
            si = inst.sync_info
            pre, post = [], []
            if si is not None and si.on_wait and len(si.on_wait) > 1:
                for w in list(si.on_wait[:-1]):
                    ctr += 1
                    pre.append(
                        mybir.InstNoOp(
                            name=f"wsplit_nop_{ctr}",
                            engine=inst.engine,
                            sync_info=mybir.SyncInfo(on_wait=[w], on_update=[]),
                        )
                    )
                si.on_wait = [si.on_wait[-1]]
            if (
                si is not None
                and si.on_update
                and len(si.on_update) > 1
                and not isinstance(inst, (mybir.InstDMACopy, mybir.InstDMA))
            ):
                for u in list(si.on_update[1:]):
                    ctr += 1
                    post.append(
                        mybir.InstNoOp(
                            name=f"usplit_nop_{ctr}",
                            engine=inst.engine,
                            sync_info=mybir.SyncInfo(on_wait=[], on_update=[u]),
                        )
                    )
                si.on_update = [si.on_update[0]]
            new_insts.extend(pre)
            new_insts.append(inst)
            new_insts.extend(post)
        if len(new_insts) != len(bb.instructions):
            bb.instructions[:] = new_insts
    for bb in fn.blocks:
        for inst in bb.instructions:
            if inst.name.startswith(("wsplit_nop_", "usplit_nop_")):
                if inst.name not in nc.inst_map:
                    nc.register_instruction(inst)
    return ctr


def _fuse_tap_ldweights(nc):
    """Tile emits one InstLdweights per matmul.  Within a tap-sweep the same
    four quadrant loads repeat for every iteration; delete an InstLdweights
    when the identical load (same tile_position, tensor, offset) is already
    resident in that quadrant, and mark every matmul ldweights=False so
    walrus doesn't re-synthesize loads for the now-bare matmuls.  Sync
    waits/updates of deleted loads are preserved on PE nops."""
    import concourse.mybir as mybir

    fn = nc.m.functions[0]
    ctr = 0
    n_deleted = 0
    for bb in fn.blocks:
        new = []
        resident = {}        # tile_position -> (memref, offset)
        changed = False
        for inst in bb.instructions:
            if isinstance(inst, mybir.InstLdweights):
                pos = tuple(inst.tile_position or (0, 0))
                ap = inst.ins[0]
                key = (ap.memref, ap.offset, tuple(tuple(d) for d in ap.ap))
                if resident.get(pos) == key:
                    si = inst.sync_info
                    if si is not None:
                        for w_ in list(si.on_wait or []):
                            ctr += 1
                            new.append(mybir.InstNoOp(
                                name=f"ldwfuse_w_{ctr}", engine=inst.engine,
                                sync_info=mybir.SyncInfo(
                                    on_wait=[w_], on_update=[]),
                            ))
                        for u in list(si.on_update or []):
                            ctr += 1
                            new.append(mybir.InstNoOp(
                                name=f"ldwfuse_u_{ctr}", engine=inst.engine,
                                sync_info=mybir.SyncInfo(
                                    on_wait=[], on_update=[u]),
                            ))
                    changed = True
                    n_deleted += 1
                    continue
                resident[pos] = key
            elif isinstance(inst, mybir.InstMatmult):
                inst.ldweights = False
            elif isinstance(inst, mybir.InstDrain):
                resident.clear()
            new.append(inst)
        if changed:
            bb.instructions[:] = new
    for bb in fn.blocks:
        for inst in bb.instructions:
            if inst.name.startswith(("ldwfuse_w_", "ldwfuse_u_")):
                if inst.name not in nc.inst_map:
                    nc.register_instruction(inst)
    return n_deleted


def build_nc():
    import concourse.bass as bass
    import concourse.mybir as mybir
    import concourse.tile as tile

    _patch_tile_drain(tile)

    f32 = mybir.dt.float32
    bf16 = mybir.dt.bfloat16
    i16 = mybir.dt.int16
    Alu = mybir.AluOpType
    Act = mybir.ActivationFunctionType

    nc = bass.Bass(trn_type="TRN2")
    xin = nc.dram_tensor("xin", (2 * P, H, W), f32, kind="ExternalInput")
    wsb = nc.dram_tensor("wsb", (P, 9 * P), bf16, kind="ExternalInput")
    sb = nc.dram_tensor("sb", (P, 2), f32, kind="ExternalInput")
    yout = nc.dram_tensor(
        "yout", (IMGS_PER_CORE, P, OBUF_N), bf16, kind="ExternalOutput"
    )

    n_pairs = IMGS_PER_CORE // 2

    # quant chunk c supplies everything slice c still needs (chunk c done =>
    # slice c complete); chunk 0 is split for a faster first slice.
    CHUNKS = [(0, 7), (8, 16)] + [
        (16 * c + 1, min(16 * c + 16, H - 1)) for c in range(1, N_SLICES)
    ]
    CHUNKS_FOR_SWEEP = [[0, 1], [2], [3], [4], [5], [6], [7]]

    with tile.TileContext(nc) as tc, ExitStack() as ctx:
        const_pool = ctx.enter_context(tc.tile_pool(name="const", bufs=1))
        xq_pool = ctx.enter_context(tc.tile_pool(name="xq", bufs=2 * N_SLICES))
        stg_pool = ctx.enter_context(tc.tile_pool(name="stg", bufs=8))
        rnd_pool = ctx.enter_context(tc.tile_pool(name="rnd", bufs=8))
        out_pool = ctx.enter_context(tc.tile_pool(name="out", bufs=8))
        obig_pool = ctx.enter_context(tc.tile_pool(name="obig", bufs=2))
        psum_pool = ctx.enter_context(
            tc.tile_pool(name="psum", bufs=8, space=bass.MemorySpace.PSUM)
        )

        w_t = const_pool.tile([P, 9 * P], bf16)
        sb_t = const_pool.tile([P, 2], f32)

        def alloc_slices():
            # slice s holds padded rows 16s .. 16s+17 (2-row halo between
            # consecutive slices); sweep s reads only slice s.
            sl = [xq_pool.tile([P, SLICE_SLOTS, HP], bf16, name="xq")
                  for _ in range(N_SLICES)]
            for s, t in enumerate(sl):
                nc.vector.memset(t[:, :, 0], 128.0)
                nc.vector.memset(t[:, :, HP - 1], 128.0)
            nc.vector.memset(sl[0][:, 0, :], 128.0)
            nc.vector.memset(sl[-1][:, SLICE_SLOTS - 1, :], 128.0)
            return sl

        def emit_quant(pr, slices, ch):
            r0, r1 = CHUNKS[ch]
            nrows = r1 - r0 + 1
            stg = stg_pool.tile([P, nrows, W], f32, name="stg")
            nc.sync.dma_start(stg[:], xin[pr * P:(pr + 1) * P, r0:r1 + 1, :])
            rnd = rnd_pool.tile([P, nrows, W], i16, name="rnd")
            # i16 = RNE(64*x + 128): hw f32->i16 cast rounds to nearest even.
            # Pair-0 chunks 2-3 round on DVE (its queue is still empty that
            # early) so they run parallel to GpSimd's chunk-1 round instead
            # of FIFO behind it -- shaves the early slice-supply gaps.
            eng = nc.vector if (pr == 0 and ch in (2, 3)) else nc.gpsimd
            eng.tensor_scalar(out=rnd[:], in0=stg[:], scalar1=64.0,
                              scalar2=128.0, op0=Alu.mult, op1=Alu.add)
            # bf16 = clip(i16, 64, 192) == x_int + 128 into the slice tiles
            # (padded rows r0+1 .. r1+1; slice s spans 16s .. 16s+17)
            p0, p1 = r0 + 1, r1 + 1
            for s in range(N_SLICES):
                lo, hi = max(p0, 16 * s), min(p1, 16 * s + SLICE_SLOTS - 1)
                if lo > hi:
                    continue
                nc.vector.tensor_scalar(
                    out=slices[s][:, lo - 16 * s:hi - 16 * s + 1, 1:1 + W],
                    in0=rnd[:, lo - p0:hi - p0 + 1, :],
                    scalar1=64, scalar2=192, op0=Alu.max, op1=Alu.min,
                )

        def emit_sweep(pr, slices, s, obig=None):
            """Conv iters 2s, 2s+1 from slice s.  Per (iter, image) one psum
            bank holds both row-blocks; redundant quadrant weight reloads are
            stripped afterwards by _fuse_tap_ldweights."""
            its = (2 * s, 2 * s + 1)
            xq = slices[s]
            ps = {}
            for it in its:
                for im in range(2):
                    ps[(it, im)] = psum_pool.tile([P, NFREE], f32, name="ps")
            for tap in range(9):
                dh, dw = divmod(tap, 3)
                first, last = tap == 0, tap == 8
                for it in its:
                    base = it * 8 - 16 * s
                    for im, r in ((0, 0), (1, 64)):
                        for blk in range(2):
                            hs = base + ROWS_PER_TILE * blk + dh
                            c = 64 * blk
                            nc.tensor.matmul(
                                ps[(it, im)][c:c + 64, :],
                                w_t[r:r + 64, tap * P + c:tap * P + c + 64],
                                xq[r:r + 64, hs:hs + ROWS_PER_TILE, dw:dw + W],
                                start=first, stop=last,
                            )
            # epilogue: one full-width op per (iter, image), ACT only (DVE
            # epilogues stall the quant-clip supply chain) -- except the
            # final two sweeps, where quant is long finished and splitting
            # image b onto DVE halves the drain tail.
            # Pair 0 accumulates into whole-image buffers flushed at its end
            # (keeps the HBM input-only while slices are being consumed);
            # pair 1 stores per-sweep so the tail stays short.
            if pr == n_pairs - 1 and s == N_SLICES - 1:
                # very last sweep: per-iteration stores, it-major, so the
                # iter-12 drain fully overlaps iter-13's matmuls.
                for j, it in enumerate(its):
                    for im in range(2):
                        obt = out_pool.tile([P, NFREE], bf16, name="obt")
                        if im == 1:
                            nc.vector.tensor_scalar(
                                out=obt[:], in0=ps[(it, im)][:],
                                scalar1=sb_t[:, 0:1], scalar2=sb_t[:, 1:2],
                                op0=Alu.mult, op1=Alu.add,
                            )
                        else:
                            nc.scalar.activation(
                                obt[:], ps[(it, im)][:], Act.Identity,
                                scale=sb_t[:, 0:1], bias=sb_t[:, 1:2],
                            )
                        # final stores split across both HWDGE rings so the
                        # two descriptor-gens run in parallel at the tail
                        dma_eng = nc.scalar if im == 1 else nc.sync
                        dma_eng.dma_start(
                            yout[2 * pr + im, :,
                                 it * NFREE:(it + 1) * NFREE], obt[:]
                        )
                return
            drain_split = pr == n_pairs - 1 and s >= N_SLICES - 2
            for im in range(2):
                if obig is not None:
                    ob, off = obig[im], its[0] * NFREE
                else:
                    ob, off = out_pool.tile([P, 2 * NFREE], bf16, name="ob"), 0
                for j, it in enumerate(its):
                    dst = ob[:, off + j * NFREE:off + (j + 1) * NFREE]
                    if im == 1 and drain_split:
                        nc.vector.tensor_scalar(
                            out=dst, in0=ps[(it, im)][:],
                            scalar1=sb_t[:, 0:1], scalar2=sb_t[:, 1:2],
                            op0=Alu.mult, op1=Alu.add,
                        )
                    else:
                        nc.scalar.activation(
                            dst, ps[(it, im)][:], Act.Identity,
                            scale=sb_t[:, 0:1], bias=sb_t[:, 1:2],
                        )
                img = 2 * pr + im
                if obig is None:
                    nc.sync.dma_start(
                        yout[img, :, its[0] * NFREE:(its[-1] + 1) * NFREE],
                        ob[:],
                    )
                elif s == N_SLICES - 1:
                    nc.sync.dma_start(yout[img, :, :], ob[:])

        # software pipeline: conv(pair k) interleaves with quant(pair k+1).
        # First input chunk's DMA goes ahead of the weight DMAs on the queue.
        slices_k = alloc_slices()
        emit_quant(0, slices_k, 0)
        nc.sync.dma_start(w_t[:], wsb[:])
        nc.sync.dma_start(sb_t[:], sb[:])
        # PE warm-up: ~40 dummy N=128 matmuls run during the initial
        # DMA/quant fill so the HAM clock gate is at 2.4 GHz (not the cold
        # 1.2 GHz) when the first real matmuls issue.
        wdum = const_pool.tile([P, P], bf16, name="wdum")
        nc.vector.memset(wdum[:], 0.0)
        psd = psum_pool.tile([P, NFREE], f32, name="ps")
        for _ in range(40):
            nc.tensor.matmul(psd[:, 0:P], wdum[:], wdum[:, 0:P],
                             start=True, stop=True)
        for ch in range(1, len(CHUNKS)):
            emit_quant(0, slices_k, ch)
        for pr in range(n_pairs):
            slices_next = alloc_slices() if pr + 1 < n_pairs else None
            obig = None
            if pr == 0:
                obig = [obig_pool.tile([P, OBUF_N], bf16, name="obig")
                        for _ in range(2)]
            for s in range(N_SLICES):
                if slices_next is not None:
                    for ch in CHUNKS_FOR_SWEEP[s]:
                        emit_quant(pr + 1, slices_next, ch)
                emit_sweep(pr, slices_k, s, obig)
            slices_k = slices_next

    _fuse_tap_ldweights(nc)
    _split_multi_syncs(nc)
    nc.finalize()
    return nc


def _host_prep(w_q, s_exp, bias, act_exp):
    """Weights: per tap a [128,128] block = 64x64 [cin,cout] duplicated 2x2
    (rows: image halves, cols: row-block halves).  Scale/bias fold."""
    w_q = np.asarray(w_q)
    w1 = np.transpose(w_q.reshape(COUT, CIN, 9), (1, 2, 0))       # [ci, t, co]
    w2 = np.concatenate([w1, w1], axis=2)                         # [ci, t, 128]
    w2 = w2.reshape(CIN, 9 * P)
    wsb = np.concatenate([w2, w2], axis=0).astype(ml_dtypes.bfloat16)

    s_exp = np.asarray(s_exp).reshape(-1).astype(np.float64)
    scale = np.exp2(float(act_exp) + s_exp)                       # [64]
    wsum = w_q.astype(np.float64).sum(axis=(1, 2, 3))             # [64]
    bias_c = np.asarray(bias).astype(np.float64) - 128.0 * wsum * scale
    col_scale = np.tile(scale, 2).astype(np.float32)
    col_bias = np.tile(bias_c, 2).astype(np.float32)
    sb = np.stack([col_scale, col_bias], axis=1)                  # [128, 2] f32
    return wsb, sb


def _decode_out(y):
    """[4, 128, 6272] bf16 -> [4, 64, 112, 112] f32.
    p = blk*64 + ch; free = it*448 + r*112 + w;
    output row = 8*it + 4*blk + r."""
    y = np.asarray(y).astype(np.float32)
    y = y.reshape(IMGS_PER_CORE, 2, CIN, N_ITERS, ROWS_PER_TILE, W)
    #            img            blk  ch   it       r             w
    y = np.transpose(y, (0, 2, 3, 1, 4, 5))      # img ch it blk r w
    return y.reshape(IMGS_PER_CORE, COUT, H, W)


def kernel(x, w_q, s_exp, bias, act_exp):
    from concourse.bass_utils import run_bass_kernel_spmd

    x = np.ascontiguousarray(np.asarray(x, dtype=np.float32))
    wsb, sb = _host_prep(np.asarray(w_q), s_exp, bias, int(act_exp))

    if "nc" not in _NC_CACHE:
        _NC_CACHE["nc"] = build_nc()
    nc = _NC_CACHE["nc"]

    in_maps = [
        {"xin": x[4 * c:4 * c + 4].reshape(2 * P, H, W), "wsb": wsb, "sb": sb}
        for c in range(N_CORES)
    ]
    _NC_CACHE["in_maps"] = in_maps
    res = run_bass_kernel_spmd(nc, in_maps, core_ids=list(range(N_CORES)))
    out = np.concatenate(
        [_decode_out(res.results[c]["yout"]) for c in range(N_CORES)], axis=0
    )
    return np.ascontiguousarray(out, dtype=np.float32)
